# revision 36
# baseline (speedup 1.0000x reference)
"""Locally-connected conv (BioConvolution) Trainium2 kernel.

Problem: Z[n,p,o] = relu(sum_{ijc} patch[n,p,i,j,c] * filt[p,i,j,c,o] + bias[o])
  X: (32,128,128,32) f32, filters: (1024,4,4,32,32) f32, bias: (32,)
  out: (32,32,32,32) f32.   FH=FW=4 non-overlapping patches, P=1024.

Sharding: patch-parallel over P across 8 cores. Core k owns patches
[128k,128k+128) == image rows [16k,16k+16). Each core touches only its own
X rows and filters: 16.8 MB in + 0.5 MB out per core — the true memory
roofline (~48 us at 358 GB/s per-core HBM; no operand is reused anywhere).

Host-side marshaling (part of sharding): the contraction axis must sit on
SBUF partitions for the PE, so X is pre-arranged per-core into
  xt[r, p, q, b] = X[b, 16k+4*pr+q, 4*pc+j, c]   (r = j*32+c, p = pr*32+pc)
and the filters into the matching ft[r, p, q, o]; both are packed into one
r-major array xf (data cols 0:32, filter cols 32:64) so every HBM->SBUF
DMA moves 128 partitions x multi-KB contiguous runs at line rate.

Device kernel (identical SPMD program on 8 cores), shipped variant fp32r:
  - All input loads issue from the sync engine's single HWDGE FIFO:
    strictly in-order chunk completions (concurrently-armed queues would
    round-robin and synchronize their completions, starving the PE), with
    a graduated [2,2,4] head so the first matmul starts early and a [4,4]
    tail to shorten the final dependency chain. bufs=8 double-buffering.
  - Per patch: 4 accumulating float32r matmuls (K=128, M=32 fout, N=32
    batch) — single-pass fp32 (~tf32 precision, rel err ~1.5e-4, half the
    PE instruction stream of true fp32 which lowers to LO/HI pairs).
    fp32r requires PSUM base partition 0, so 8 patches pack side-by-side
    along the free axis of one PSUM bank [32, 8x32].
  - ScalarE applies bias+ReLU per PSUM bank into an SBUF staging buffer;
    output stores ride ScalarE's own HWDGE ring LAGGED two groups behind
    the ACT stream, so their dependencies are long complete and they can
    never head-of-line block either the load FIFO or the ACT stream
    (gpsimd/SWDGE stores were tried and added multi-us Q7 drain jitter).
  - Two 4-patch mini-groups at the end halve the final
    load->matmul->ACT->store dependency chain.
Measured: ~62-66 us NEFF exec across runs (~±2 us device jitter), vs a
~48 us pure-traffic roofline at the 358 GB/s per-core HBM wall; ~8.7 us
is fixed engine-boot/Tile-preamble before the first DMA packet can flow,
~4 us is the unavoidable tail (final chain + store completion + Tile
drain barrier).
"""

import numpy as np

N, H, W, C = 32, 128, 128, 32
FH = FW = 4
FOUT = 32
NCORES = 8
PL = 128          # patches per core
NQ = 4            # K-chunks per patch (512 / 128)
KR = 128          # contraction rows per chunk (SBUF partitions)
NG = PL // 4      # 4-patch groups per core

_CACHE = {}


def _build_module(bufs=6, out_splits=8, mm_dtype="float32"):
    from concourse import bacc, tile, mybir

    nc = bacc.Bacc("TRN2", target_bir_lowering=False, debug=False, enable_asserts=False)
    dt = mybir.dt.float32
    mdt = getattr(mybir.dt, mm_dtype)
    # xf packs data and filters: [..., 0:32] = batch cols, [..., 32:64] = fout
    xf = nc.dram_tensor("xf", [KR, PL, NQ, N + FOUT], mdt, kind="ExternalInput").ap()
    bt = nc.dram_tensor("bt", [KR, 1], dt, kind="ExternalInput").ap()
    out = nc.dram_tensor("out", [KR, NG, N], dt, kind="ExternalOutput").ap()

    # Graduated chunk sizes (in patches): small first chunks so the first
    # matmul isn't gated on a full-size load sharing bandwidth round-robin.
    sizes = [2, 2, 4]
    rest = PL - sum(sizes)
    sizes += [8] * (rest // 8)
    assert sum(sizes) == PL
    GSPLIT = NG // out_splits
    relu = mybir.ActivationFunctionType.Relu

    with tile.TileContext(nc) as tc:
        with (
            tc.tile_pool(name="xfpool", bufs=bufs) as xfpool,
            tc.tile_pool(name="psum", bufs=8, space="PSUM") as psum,
            tc.tile_pool(name="misc", bufs=1) as misc,
        ):
            bias_t = misc.tile([KR, 1], dt)
            nc.sync.dma_start(bias_t[:], bt[:])
            staging = misc.tile([KR, NG, N], dt)

            p0 = 0
            for ch, PC in enumerate(sizes):
                xtile = xfpool.tile([KR, PC, NQ, N + FOUT], mdt, tag="xf")
                sl = slice(p0, p0 + PC)
                eng = nc.sync if ch % 2 == 0 else nc.scalar
                eng.dma_start(xtile[:], xf[:, sl, :, :])
                for g in range(PC // 2):
                    gg = (p0 + g * 2) // 4       # psum group id (2 patches/iter)
                    half = (p0 + g * 2) % 4      # 0 or 2: which half of the group
                    if half == 0:
                        ptile = psum.tile([KR, N], dt, tag="ps")
                    for s2 in range(2):
                        s = half + s2
                        p = g * 2 + s2
                        for q in range(NQ):
                            nc.tensor.matmul(
                                ptile[32 * s : 32 * s + 32, :],
                                xtile[:, p, q, N : N + FOUT],  # lhsT [128,32(o)]
                                xtile[:, p, q, 0:N],           # rhs  [128,32(b)]
                                start=(q == 0),
                                stop=(q == NQ - 1),
                                tile_position=(0, 32 * s),
                            )
                    if half == 2:
                        nc.scalar.activation(
                            staging[:, gg, :], ptile[:], relu, bias=bias_t[:]
                        )
                        if (gg + 1) % GSPLIT == 0:
                            osl = slice(gg + 1 - GSPLIT, gg + 1)
                            oeng = nc.sync if gg + 1 == NG else nc.gpsimd
                            oeng.dma_start(out[:, osl, :], staging[:, osl, :])
                p0 += PC
    nc.compile()
    return nc


def _build_module_r(bufs=8):
    """float32r variant: single-pass fp32 matmuls (tf32-ish precision),
    PSUM packing along the free axis (8 patches per bank) since fp32r
    requires dst base partition 0. Half the PE instruction stream of the
    fp32 variant -> fewer IRAM paging stalls."""
    from concourse import bacc, tile, mybir

    nc = bacc.Bacc("TRN2", target_bir_lowering=False, debug=False, enable_asserts=False)
    dt = mybir.dt.float32
    mdt = mybir.dt.float32r
    SG = 8                      # patches per PSUM super-group
    NSG = PL // SG              # 16
    xf = nc.dram_tensor("xf", [KR, PL, NQ, N + FOUT], mdt, kind="ExternalInput").ap()
    bt = nc.dram_tensor("bt", [FOUT, 1], dt, kind="ExternalInput").ap()
    out = nc.dram_tensor("out", [FOUT, PL, N], dt, kind="ExternalOutput").ap()

    # Graduated [2,2,4] head (earliest first matmul; measured tightest
    # variance) and a [4,4] tail that halves the final
    # load->matmul->ACT->store chain.
    sizes = [2, 2, 4] + [8] * ((PL - 16) // 8) + [4, 2, 2]
    assert sum(sizes) == PL
    # PSUM eviction groups: 8-patch banks, except two 4-patch mini-groups
    # at the end so the last matmul->ACT->store chain is half as long.
    groups = [(g * SG, SG) for g in range(NSG - 1)] + [(PL - 8, 4), (PL - 4, 4)]
    gof = {}
    for gi, (s0, gsz) in enumerate(groups):
        for i in range(gsz):
            gof[s0 + i] = (gi, i)
    relu = mybir.ActivationFunctionType.Relu

    with tile.TileContext(nc) as tc:
        with (
            tc.tile_pool(name="xfpool", bufs=bufs) as xfpool,
            tc.tile_pool(name="psum", bufs=6, space="PSUM") as psum,
            tc.tile_pool(name="misc", bufs=1) as misc,
        ):
            # bias rides the scalar ring so it doesn't burn sync's first
            # DMA slot (~0.7 us of stream start).
            bias_t = misc.tile([FOUT, 1], dt)
            nc.scalar.dma_start(bias_t[:], bt[:])
            staging = misc.tile([FOUT, PL, N], dt)

            p0 = 0
            ptile = None
            for ch, PC in enumerate(sizes):
                xtile = xfpool.tile([KR, PC, NQ, N + FOUT], mdt, tag="xf")
                # All loads on sync's single HWDGE FIFO: strictly in-order
                # completions. (Arming chunk 0 on the scalar ring was tried
                # and is bimodal: when sync's big queue gets ahead, chunk 0
                # drains at round-robin half-rate and the in-order PE
                # consumption slips ~8 us.)
                nc.sync.dma_start(xtile[:], xf[:, p0 : p0 + PC, :, :])
                for pl in range(PC):
                    p = p0 + pl
                    gi, i = gof[p]
                    s0, gsz = groups[gi]
                    if i == 0:
                        ptile = psum.tile([FOUT, SG, N], dt, tag="ps")
                    for q in range(NQ):
                        nc.tensor.matmul(
                            ptile[:, i, :],
                            xtile[:, pl, q, N : N + FOUT],  # lhsT [128,32(o)]
                            xtile[:, pl, q, 0:N],           # rhs  [128,32(b)]
                            start=(q == 0),
                            stop=(q == NQ - 1),
                        )
                    if i == gsz - 1:
                        nc.scalar.activation(
                            staging[:, s0 : s0 + gsz, :],
                            ptile[:, :gsz, :],
                            relu,
                            bias=bias_t[:],
                        )
                        # Stores also ride the scalar ring, LAGGED two groups
                        # behind the ACT stream: their ACT dependency is long
                        # complete, so they never stall scalar (and the sync
                        # load ring is untouched). The final two stores are
                        # pure program-order after the last ACT.
                        if gi == len(groups) - 1:
                            a = groups[gi - 2][0]
                            nc.scalar.dma_start(
                                out[:, a:s0, :], staging[:, a:s0, :]
                            )
                            nc.scalar.dma_start(
                                out[:, s0:PL, :], staging[:, s0:PL, :]
                            )
                        elif gi % 2 == 1 and gi >= 3:
                            a = groups[gi - 3][0]
                            b = groups[gi - 1][0]
                            nc.scalar.dma_start(
                                out[:, a:b, :], staging[:, a:b, :]
                            )
                p0 += PC
    nc.compile()
    return nc


def _build_module_bf16(bufs=6):
    """bf16 variant: inputs cast to bf16 host-side (halves HBM traffic, the
    true bottleneck; fp32 PSUM accumulation keeps rel err ~1e-3). Matmuls
    are column-tiled: 4 patches' [128,32] filter blocks sit on the 4
    column strips of the PE array via tile_position=(0,32s), emitted
    q-outer so the 4 strips run concurrently (per-subarray concurrency)
    and the PE stream can't pace the now-faster load stream."""
    from concourse import bacc, tile, mybir

    nc = bacc.Bacc("TRN2", target_bir_lowering=False, debug=False, enable_asserts=False)
    dt = mybir.dt.float32
    mdt = mybir.dt.bfloat16
    xf = nc.dram_tensor("xf", [KR, PL, NQ, N + FOUT], mdt, kind="ExternalInput").ap()
    bt = nc.dram_tensor("bt", [KR, 1], dt, kind="ExternalInput").ap()
    # Output stored bf16 (host upcasts): halves store traffic and staging.
    out = nc.dram_tensor("out", [KR, NG, N], mdt, kind="ExternalOutput").ap()

    # Graduated head (earliest first matmul) and a short tail that
    # shortens the final load->matmul->ACT->store chain. 16-patch main
    # chunks keep per-arm transfer (~1.4 us) above the sync engine's
    # ~0.65 us arm rate and halve descriptor volume vs 8.
    sizes = [2, 2, 4, 8] + [16] * 6 + [8, 4, 2, 2]
    assert sum(sizes) == PL
    # ACT groups: 8 patches (two 4-patch col-tile passes side by side on
    # the PSUM free axis), except two 4-patch mini-groups at the end.
    groups = [(g * 8, 8) for g in range((PL - 8) // 8)] + [(PL - 8, 4), (PL - 4, 4)]
    gof = {}
    for gi, (s0, gsz) in enumerate(groups):
        for i in range(gsz):
            gof[s0 + i] = gi
    relu = mybir.ActivationFunctionType.Relu

    with tile.TileContext(nc) as tc:
        with (
            # The whole per-core input (64 KB/partition) fits in SBUF, so
            # every chunk gets its own buffer: load arms carry no
            # buffer-recycle waits and all issue back-to-back at the start.
            tc.tile_pool(name="xfpool", bufs=len(sizes)) as xfpool,
            # One PSUM tile per ACT group (17 x 256B/partition): no PSUM
            # recycling, so matmul groups never stall on an ACT 8 groups
            # back (the recycle coupling cost multi-us hiccups).
            tc.tile_pool(name="psum", bufs=17, space="PSUM") as psum,
            tc.tile_pool(name="misc", bufs=1) as misc,
        ):
            bias_t = misc.tile([KR, 1], dt)
            nc.scalar.dma_start(bias_t[:], bt[:])
            staging = misc.tile([KR, NG, N], mdt)

            # All load arms first, in program order: any completion-sem
            # reuse then only ever waits on an earlier LOAD (long done) —
            # never on a store, whose packets drain at a trickle behind
            # the load queue and would stall the whole load stream.
            xtiles = []
            p0 = 0
            for PC in sizes:
                xtile = xfpool.tile([KR, PC, NQ, N + FOUT], mdt, tag="xf")
                nc.sync.dma_start(xtile[:], xf[:, p0 : p0 + PC, :, :])
                xtiles.append((p0, PC, xtile))
                p0 += PC

            ptile = None
            for p0, PC, xtile in xtiles:
                # Matmuls go q-outer within each 4-patch col-tile pass so
                # consecutive instructions hit different column strips.
                for b0 in range(0, PC, 4):
                    npass = min(4, PC - b0)        # patches in this pass
                    p = p0 + b0
                    gi = gof[p]
                    s0, gsz = groups[gi]
                    j = (p - s0) // 4              # free-axis slot in psum tile
                    if p == s0:
                        ptile = psum.tile([KR, gsz // 4, N], dt, tag="ps")
                    for q in range(NQ):
                        for si in range(npass):
                            s = (p + si - s0) % 4
                            nc.tensor.matmul(
                                ptile[32 * s : 32 * s + 32, j, :],
                                xtile[:, b0 + si, q, N : N + FOUT],
                                xtile[:, b0 + si, q, 0:N],
                                start=(q == 0),
                                stop=(q == NQ - 1),
                                tile_position=(0, 32 * s),
                            )
                    if p + npass == s0 + gsz:
                        g4 = s0 // 4
                        ng4 = gsz // 4
                        nc.scalar.activation(
                            staging[:, g4 : g4 + ng4, :],
                            ptile[:, :ng4, :],
                            relu,
                            bias=bias_t[:],
                        )
                        # Two stores at the end, on sync's queue behind the
                        # loads (in-order: packets flow the moment the load
                        # stream drains). Arm the bulk store a few groups
                        # early so its descriptor fetch overlaps the
                        # remaining loads; the last arm covers only the
                        # final 4 groups to keep the tail chain short.
                        if gi == len(groups) - 4:
                            a = g4 + ng4
                            nc.sync.dma_start(
                                out[:, 0:a, :], staging[:, 0:a, :]
                            )
                        elif gi == len(groups) - 1:
                            a = groups[len(groups) - 4][0] // 4 + 2
                            e = PL // 4
                            nc.sync.dma_start(
                                out[:, a:e, :], staging[:, a:e, :]
                            )
    nc.compile()
    return nc


def _build_module_ct2():
    """Shared-LDWEIGHTS col-tiled variant. The tensor engine's sustained
    issue rate is ~34 ns/instruction (NX fetch-limited; bursts only drain
    the 64-deep queue), so the 1024-instruction LDW+MM stream of the
    per-patch variant paces the whole kernel. Here ONE 128-column
    LDWEIGHTS per (4-patch group, q) loads all 4 strips' filters and the
    4 matmuls are emitted non-self-loading (InstMatmult.ldweights=False):
    640 tensor instructions. A ~3.4 us warm-up primer of dummy matmuls at
    program start flips the PE's HAM clock gate to 2.4 GHz before the
    real stream begins."""
    from concourse import bacc, tile, mybir

    nc = bacc.Bacc("TRN2", target_bir_lowering=False, debug=False, enable_asserts=False)
    dt = mybir.dt.float32
    mdt = mybir.dt.bfloat16
    xf = nc.dram_tensor("xf", [KR, PL, NQ, N + FOUT], mdt, kind="ExternalInput").ap()
    bt = nc.dram_tensor("bt", [KR, 1], dt, kind="ExternalInput").ap()
    out = nc.dram_tensor("out", [KR, NG, N], mdt, kind="ExternalOutput").ap()

    sizes = [2, 2, 4, 8] + [16] * 6 + [8, 4, 2, 2]
    assert sum(sizes) == PL
    groups = [(g * 8, 8) for g in range((PL - 8) // 8)] + [(PL - 8, 4), (PL - 4, 4)]
    gof = {}
    for gi, (s0, gsz) in enumerate(groups):
        for i in range(gsz):
            gof[s0 + i] = gi
    relu = mybir.ActivationFunctionType.Relu

    with tile.TileContext(nc) as tc:
        with (
            tc.tile_pool(name="xfpool", bufs=len(sizes)) as xfpool,
            tc.tile_pool(name="psum", bufs=7, space="PSUM") as psum,
            tc.tile_pool(name="pscr", bufs=1, space="PSUM") as pscr,
            tc.tile_pool(name="misc", bufs=1) as misc,
        ):
            bias_t = misc.tile([KR, 1], dt)
            nc.scalar.dma_start(bias_t[:], bt[:])
            staging = misc.tile([KR, NG, N], mdt)

            # HAM warm-up primer: ~9 zero x zero matmuls keep the PE array
            # busy from program start (~6.7 us) until real data arrives
            # (~10.3 us) so the clock gate is at 8/8 for the whole stream.
            scratch = misc.tile([KR, 640], mdt)
            nc.gpsimd.memset(scratch[:], 0)
            pdummy = pscr.tile([KR, 512], dt)
            for _ in range(9):
                nc.tensor.matmul(
                    pdummy[:],
                    scratch[:, 512:640],
                    scratch[:, 0:512],
                    start=True,
                    stop=True,
                    skip_group_check=True,
                )

            xtiles = []
            p0 = 0
            for PC in sizes:
                xtile = xfpool.tile([KR, PC, NQ, N + FOUT], mdt, tag="xf")
                nc.sync.dma_start(xtile[:], xf[:, p0 : p0 + PC, :, :])
                xtiles.append((p0, PC, xtile))
                p0 += PC

            ptile = None
            for p0, PC, xtile in xtiles:
                for b0 in range(0, PC, 4):
                    npass = min(4, PC - b0)
                    p = p0 + b0
                    gi = gof[p]
                    s0, gsz = groups[gi]
                    j = (p - s0) // 4
                    if p == s0:
                        ptile = psum.tile([KR, gsz // 4, N], dt, tag="ps")
                    s_start = (p - s0) % 4
                    for q in range(NQ):
                        # One LDW covers this pass's patches across the
                        # column strips (columns 32*s_start + p_local*32+o).
                        nc.tensor.ldweights(
                            xtile[:, b0 : b0 + npass, q, N : N + FOUT],
                            tile_position=(0, 32 * s_start),
                        )
                        for si in range(npass):
                            s = (p + si - s0) % 4
                            mm = nc.tensor.matmul(
                                ptile[32 * s : 32 * s + 32, j, :],
                                xtile[:, b0 + si, q, N : N + FOUT],
                                xtile[:, b0 + si, q, 0:N],
                                start=(q == 0),
                                stop=(q == NQ - 1),
                                tile_position=(0, 32 * s),
                                skip_group_check=True,
                            )
                            mm.ins.ldweights = False
                    if p + npass == s0 + gsz:
                        g4 = s0 // 4
                        ng4 = gsz // 4
                        nc.scalar.activation(
                            staging[:, g4 : g4 + ng4, :],
                            ptile[:, :ng4, :],
                            relu,
                            bias=bias_t[:],
                        )
                        if gi == len(groups) - 4:
                            a = g4 + ng4
                            nc.sync.dma_start(
                                out[:, 0:a, :], staging[:, 0:a, :]
                            )
                        elif gi == len(groups) - 1:
                            a = groups[len(groups) - 4][0] // 4 + 2
                            e = PL // 4
                            nc.sync.dma_start(
                                out[:, a:e, :], staging[:, a:e, :]
                            )
    nc.compile()
    return nc


def _build_module_bd2():
    """2-patch-block variant. The tensor engine's sustained issue rate is
    ~34 ns/instruction (NX fetch-limited), so the 1024-instruction
    per-patch stream (512 self-loading matmuls) paces the kernel at
    ~35 us. Here each matmul covers TWO patches: stationary = filters of
    2 patches [128, (p,o)=64], moving = X of 2 patches [128, (p',b)=64],
    out = [64, 64] of which the two diagonal 32x32 blocks are wanted Z.
    Two blocks per 4-patch group sit at column positions 0/64 and overlap
    in the array. 256 matmuls = 512 tensor fetches. Bias+ReLU runs in the
    diagonal extraction: scalar ACT (bias AP + Relu) for the lower half,
    DVE tensor_scalar(add bias, max 0) for the upper half."""
    from concourse import bacc, tile, mybir

    nc = bacc.Bacc("TRN2", target_bir_lowering=False, debug=False, enable_asserts=False)
    dt = mybir.dt.float32
    mdt = mybir.dt.bfloat16
    NB = PL // 2                       # 2-patch blocks per core
    # xf cols per (block, q): [0:64] = X of the 2 patches ((w,b) flat),
    # [64:128] = filters ((w,o) flat) — contiguous, single-free-dim APs.
    xf = nc.dram_tensor("xf", [KR, NB, NQ, 128], mdt, kind="ExternalInput").ap()
    bt = nc.dram_tensor("bt", [KR, 1], dt, kind="ExternalInput").ap()
    out = nc.dram_tensor("out", [KR, NG, N], mdt, kind="ExternalOutput").ap()

    sizes = [1, 1, 2, 4] + [8] * 6 + [4, 2, 2]      # in blocks
    assert sum(sizes) == NB
    # PSUM groups: 8 groups of 8 blocks = 8 tiles = all 8 banks, so the
    # pool NEVER recycles a bank. This is load-bearing for correctness:
    # Tile's pool-recycle waits only track the scalar reader of a PSUM
    # tile (verified in the IR), so any recycled bank's matmuls could
    # race the vector extraction.
    relu = mybir.ActivationFunctionType.Relu
    add_op = mybir.AluOpType.add
    max_op = mybir.AluOpType.max

    with tile.TileContext(nc) as tc:
        with (
            tc.tile_pool(name="xfpool", bufs=len(sizes)) as xfpool,
            # ONE psum pool shared by both extraction engines. Tile chains
            # cross-engine readers of a pooled tile (serializing vector
            # extractions ~300ns behind scalar's) — that chain is what
            # makes the pool-recycle waits sound: with per-engine pools the
            # recycled banks' matmuls carried NO wait on the DVE reader
            # (verified in the IR) and intermittently raced it.
            tc.tile_pool(name="psum", bufs=8, space="PSUM") as psum,
            tc.tile_pool(name="misc", bufs=1) as misc,
        ):
            bias_t = misc.tile([KR, 1], dt)
            nc.scalar.dma_start(bias_t[:], bt[:])
            staging = misc.tile([KR, NG, N], mdt)

            xtiles = []
            b0 = 0
            for BC in sizes:
                xtile = xfpool.tile([KR, BC, NQ, 128], mdt, tag="xf")
                nc.sync.dma_start(xtile[:], xf[:, b0 : b0 + BC, :, :])
                xtiles.append((b0, BC, xtile))
                b0 += BC

            ptile = None

            def extract(gi, ptile, j_lo, j_hi):
                # Diagonal extraction + bias + ReLU for psum slots
                # [j_lo, j_hi) of group gi; scalar takes the lower block
                # half, DVE the upper.
                for h in range(2):
                    for w in range(2):
                        pl4 = 2 * h + w
                        pr = 64 * h + 32 * w
                        src = ptile[pr : pr + 32, j_lo:j_hi, 32 * w : 32 * w + 32]
                        dst = staging[
                            32 * pl4 : 32 * pl4 + 32,
                            4 * gi + j_lo : 4 * gi + j_hi,
                            :,
                        ]
                        bsl = bias_t[pr : pr + 32]
                        if h == 0:
                            nc.scalar.activation(dst, src, relu, bias=bsl)
                        else:
                            nc.vector.tensor_scalar(
                                dst, src, bsl, 0.0, add_op, max_op
                            )

            for c0, BC, xtile in xtiles:
                for j0 in range(0, BC, 2):
                    npair = min(2, BC - j0)
                    blk0 = c0 + j0
                    gi = blk0 // 8                 # psum group (bank)
                    j = (blk0 % 8) // 2            # psum free slot (pair idx)
                    if blk0 % 8 == 0:
                        ptile = psum.tile([KR, 4, 64], dt, tag="ps")
                    # q-outer, block-inner: the pair's matmuls overlap in
                    # the array (column positions 0/64) so the chain is
                    # ~half as long as two serial per-block chains.
                    for q in range(NQ):
                        for h in range(npair):
                            blk = blk0 + h
                            off = 64 * (blk % 2)   # block position: 0 or 64
                            nc.tensor.matmul(
                                ptile[off : off + 64, j, :],
                                xtile[:, j0 + h, q, 64:128],  # stationary: filters
                                xtile[:, j0 + h, q, 0:64],    # moving: X
                                start=(q == 0),
                                stop=(q == NQ - 1),
                                tile_position=(0, off),
                                skip_group_check=True,
                            )
                    blk = blk0 + npair - 1
                    # Two-stage extraction keeps the final chain short: the
                    # first 3 slots drain as soon as their matmuls finish,
                    # only the last pair's slot remains after the last MM.
                    if blk % 8 == 5:
                        extract(gi, ptile, 0, 3)
                    elif blk % 8 == 7:
                        extract(gi, ptile, 3, 4)
                        if gi == 6:
                            nc.sync.dma_start(
                                out[:, 0:28, :], staging[:, 0:28, :]
                            )
                        elif gi == 7:
                            # Final store on the scalar ring: its queue is
                            # empty, so descriptor fetch and packets
                            # overlap the bulk store draining behind Q1.
                            nc.scalar.dma_start(
                                out[:, 28:32, :], staging[:, 28:32, :]
                            )
    nc.compile()
    return nc


def _build_module_bd():
    """Block-diagonal variant: per 4-patch group and K-chunk q, ONE
    128-column LDWEIGHTS loads the 4 patches' X [128, (p,b)=128] as the
    stationary operand and ONE matmul streams the 4 patches' filters
    [128, (p',o)=128] as the moving operand, computing all 16 cross
    blocks out[(p,b),(p',o)] of which the 4 diagonal p==p' blocks are the
    wanted Z. 8 tensor instructions per group (~280 total) instead of 32
    (~1024): the tensor stream was issue-limited at ~34 ns/instruction,
    not FLOP-limited, and 4x fewer instructions takes it off the critical
    path. Bias is folded in as a K=1 rank-one matmul (ones x bias) so the
    diagonal extraction is a pure ReLU, split across the scalar AND
    vector engines (2 strips each)."""
    from concourse import bacc, tile, mybir

    nc = bacc.Bacc("TRN2", target_bir_lowering=False, debug=False, enable_asserts=False)
    dt = mybir.dt.float32
    mdt = mybir.dt.bfloat16
    NG4 = PL // 4                      # 4-patch groups per core
    # xf2 cols: [0:128] = X block (p_local*32+b), [128:256] = filter block
    # (p_local*32+o), both contiguous so LDWEIGHTS gets FWL and the moving
    # stream is a single run.
    xf = nc.dram_tensor("xf", [KR, NG4, NQ, 256], mdt, kind="ExternalInput").ap()
    # bt: [0:128] = ones (K=1 stationary for the bias matmul),
    # [128:640] = bias tiled 16x (moving operand, 128 per group slot).
    bt = nc.dram_tensor("bt", [1, 640], mdt, kind="ExternalInput").ap()
    out = nc.dram_tensor("out", [KR, NG4, FOUT], dt, kind="ExternalOutput").ap()

    # Chunk sizes in 4-patch groups; short head and tail chunks keep the
    # first matmul early and the final load->mm->relu->store chain short.
    sizes = [1, 1, 2] + [4] * 6 + [2, 1, 1]
    assert sum(sizes) == NG4
    relu = mybir.ActivationFunctionType.Relu

    with tile.TileContext(nc) as tc:
        with (
            tc.tile_pool(name="xfpool", bufs=len(sizes)) as xfpool,
            tc.tile_pool(name="psum", bufs=4, space="PSUM") as psum,
            tc.tile_pool(name="misc", bufs=1) as misc,
        ):
            bias_t = misc.tile([1, 640], mdt)
            nc.scalar.dma_start(bias_t[:], bt[:])
            staging = misc.tile([KR, NG4, FOUT], dt)

            # All load arms first (whole input resident in SBUF): no
            # buffer-recycle waits, and completion-sem reuse only ever
            # references an earlier load.
            xtiles = []
            g0 = 0
            for GC in sizes:
                xtile = xfpool.tile([KR, GC, NQ, 256], mdt, tag="xf")
                nc.sync.dma_start(xtile[:], xf[:, g0 : g0 + GC, :, :])
                xtiles.append((g0, GC, xtile))
                g0 += GC

            for ci, (g0, GC, xtile) in enumerate(xtiles):
                ptile = psum.tile([KR, GC, 128], dt, tag="ps")
                for j in range(GC):
                    for q in range(NQ):
                        nc.tensor.matmul(
                            ptile[:, j, :],
                            xtile[:, j, q, 0:128],      # stationary: X
                            xtile[:, j, q, 128:256],    # moving: filters
                            start=(q == 0),
                            stop=False,
                            skip_group_check=True,
                        )
                    # K=1 matmul adds bias[o] to every column of region j.
                    # Must come before the NEXT region's start=True matmul:
                    # start clears has_written beyond its own out region,
                    # and a start=False write onto cleared bits OVERWRITES
                    # instead of accumulating.
                    nc.tensor.matmul(
                        ptile[:, j, :],
                        bias_t[:, 0:128],
                        bias_t[:, 128:256],
                        start=False,
                        stop=True,
                        skip_group_check=True,
                    )
                # Diagonal extraction + ReLU, 2 strips on scalar, 2 on DVE.
                for s in range(4):
                    src = ptile[32 * s : 32 * s + 32, :, 32 * s : 32 * s + 32]
                    dst = staging[32 * s : 32 * s + 32, g0 : g0 + GC, :]
                    if s < 2:
                        nc.scalar.activation(dst, src, relu)
                    else:
                        nc.vector.tensor_scalar_max(dst, src, 0.0)
                if ci == len(sizes) - 2:
                    a = g0 + GC
                    nc.sync.dma_start(out[:, 0:a, :], staging[:, 0:a, :])
                elif ci == len(sizes) - 1:
                    a = g0
                    nc.sync.dma_start(out[:, a:NG4, :], staging[:, a:NG4, :])
    nc.compile()
    return nc


def _get_module():
    if "nc" not in _CACHE:
        _CACHE["nc"] = _build_module()
    return _CACHE["nc"]


def _marshal(X, filters, bias, mdtype=np.float32):
    """Shard + lay out full inputs into per-core device arrays."""
    X = np.ascontiguousarray(np.asarray(X, dtype=np.float32))
    filters = np.ascontiguousarray(np.asarray(filters, dtype=np.float32))
    bias = np.asarray(bias, dtype=np.float32)

    # X: (b, core, pr, i, pc, j, c) -> (core, j, c, pr, pc, i, b)
    xv = X.reshape(N, NCORES, 4, FH, 32, FW, C)
    xt = xv.transpose(1, 5, 6, 2, 4, 3, 0).reshape(NCORES, KR, PL, NQ, N)
    # filters: (core, p, i, j, c, o) -> (core, j, c, p, i, o)
    fv = filters.reshape(NCORES, PL, FH, FW, C, FOUT)
    ft = fv.transpose(0, 3, 4, 1, 2, 5).reshape(NCORES, KR, PL, NQ, FOUT)
    xfa = np.concatenate([xt, ft], axis=4)
    if xfa.dtype != mdtype:
        xfa = xfa.astype(mdtype)  # round-to-nearest-even
    xfa = np.ascontiguousarray(xfa)
    bt = np.ascontiguousarray(np.tile(bias, 4).reshape(KR, 1))
    return xfa, bt


def _assemble(outs):
    """Per-core out [128=(s,o), NG, N] -> full (N, 32, 32, FOUT)."""
    z = np.stack(outs).astype(np.float32)               # (core, (s,o), g, b)
    z = z.reshape(NCORES, 4, FOUT, NG, N)               # (core, s, o, g, b)
    z = z.transpose(4, 0, 3, 1, 2)                      # (b, core, g, s, o)
    z = z.reshape(N, NCORES, PL, FOUT)                  # p_loc = 4*g + s
    z = z.reshape(N, NCORES * 4, 32, FOUT)              # (b, pr_glob, pc, o)
    return np.ascontiguousarray(z)


def _assemble_r(outs):
    """Per-core out [FOUT, PL, N] -> full (N, 32, 32, FOUT)."""
    z = np.stack(outs)                                  # (core, o, p, b)
    z = z.transpose(3, 0, 2, 1)                         # (b, core, p, o)
    return np.ascontiguousarray(z.reshape(N, 32, 32, FOUT))


def _marshal_bd2(X, filters, bias):
    """Pair-blocked layout for bd2: xf[r, blk, q, 0:64] = X of block blk's
    2 patches ((w,b) flattened), [.., 64:128] = the matching filter block
    ((w,o) flattened); both contiguous (single-free-dim matmul APs)."""
    import ml_dtypes

    bf16 = ml_dtypes.bfloat16
    NB = PL // 2
    X = np.ascontiguousarray(np.asarray(X, dtype=np.float32))
    filters = np.ascontiguousarray(np.asarray(filters, dtype=np.float32))
    bias = np.asarray(bias, dtype=np.float32)

    xv = X.reshape(N, NCORES, 4, FH, 32, FW, C)
    xt = xv.transpose(1, 5, 6, 2, 4, 3, 0).reshape(NCORES, KR, PL, NQ, N)
    xt = xt.reshape(NCORES, KR, NB, 2, NQ, N).transpose(0, 1, 2, 4, 3, 5)
    xt = xt.reshape(NCORES, KR, NB, NQ, 64)
    fv = filters.reshape(NCORES, PL, FH, FW, C, FOUT)
    ft = fv.transpose(0, 3, 4, 1, 2, 5).reshape(NCORES, KR, PL, NQ, FOUT)
    ft = ft.reshape(NCORES, KR, NB, 2, NQ, FOUT).transpose(0, 1, 2, 4, 3, 5)
    ft = ft.reshape(NCORES, KR, NB, NQ, 64)
    xfa = np.ascontiguousarray(np.concatenate([xt, ft], axis=4).astype(bf16))
    bt = np.ascontiguousarray(np.tile(bias, 4).reshape(KR, 1))
    return xfa, bt


def _marshal_bd(X, filters, bias):
    """Block-diagonal layout: xf2[r, g, q, 0:128] = X of group g's 4
    patches ((p_local, b) flattened), [.., 128:256] = the matching filter
    block ((p_local, o) flattened); bt = [ones(128), tile(bias, 16)]."""
    import ml_dtypes

    bf16 = ml_dtypes.bfloat16
    X = np.ascontiguousarray(np.asarray(X, dtype=np.float32))
    filters = np.ascontiguousarray(np.asarray(filters, dtype=np.float32))
    bias = np.asarray(bias, dtype=np.float32)

    NG4 = PL // 4
    # X: (b, core, pr, i, pc, j, c) -> (core, j, c, pr, pc, i, b)
    xv = X.reshape(N, NCORES, 4, FH, 32, FW, C)
    xt = xv.transpose(1, 5, 6, 2, 4, 3, 0).reshape(NCORES, KR, PL, NQ, N)
    # -> (core, r, g, q, p_local, b)
    xt = xt.reshape(NCORES, KR, NG4, 4, NQ, N).transpose(0, 1, 2, 4, 3, 5)
    xt = xt.reshape(NCORES, KR, NG4, NQ, 128)
    fv = filters.reshape(NCORES, PL, FH, FW, C, FOUT)
    ft = fv.transpose(0, 3, 4, 1, 2, 5).reshape(NCORES, KR, PL, NQ, FOUT)
    ft = ft.reshape(NCORES, KR, NG4, 4, NQ, FOUT).transpose(0, 1, 2, 4, 3, 5)
    ft = ft.reshape(NCORES, KR, NG4, NQ, 128)
    xfa = np.ascontiguousarray(
        np.concatenate([xt, ft], axis=4).astype(bf16)
    )
    bt = np.ascontiguousarray(
        np.concatenate([np.ones(128, np.float32), np.tile(bias, 16)])
        .astype(bf16)
        .reshape(1, 640)
    )
    return xfa, bt


def _assemble_bd(outs):
    """Per-core out [(s,b)=128, NG4, FOUT] -> full (N, 32, 32, FOUT)."""
    NG4 = PL // 4
    z = np.stack(outs)                                  # (core, (s,b), g, o)
    z = z.reshape(NCORES, 4, N, NG4, FOUT)              # (core, s, b, g, o)
    z = z.transpose(2, 0, 3, 1, 4)                      # (b, core, g, s, o)
    z = z.reshape(N, NCORES, PL, FOUT)                  # p_loc = 4*g + s
    return np.ascontiguousarray(z.reshape(N, NCORES * 4, 32, FOUT))


LAST_RESULT = None
VARIANT = "bd2"


def kernel(X, filters, bias):
    global LAST_RESULT
    from concourse import bass_utils
    from concourse.bass_utils import run_bass_kernel_spmd

    # If tracing is enabled in the environment, keep the artifact upload
    # local so a missing bucket can't fail the run.
    bass_utils.upload_artifacts = lambda tmpdir: f"local://{tmpdir}"

    if "nc" not in _CACHE:
        _CACHE["nc"] = {
            "fp32r": _build_module_r,
            "fp32": _build_module,
            "bf16ct": _build_module_bf16,
            "ct2": _build_module_ct2,
            "bd2": _build_module_bd2,
            "bd": _build_module_bd,
        }[VARIANT]()
    nc = _CACHE["nc"]
    if VARIANT == "bd":
        xfa, bt = _marshal_bd(X, filters, bias)
    elif VARIANT == "bd2":
        xfa, bt = _marshal_bd2(X, filters, bias)
    elif VARIANT in ("bf16ct", "ct2"):
        import ml_dtypes

        xfa, bt = _marshal(X, filters, bias, mdtype=ml_dtypes.bfloat16)
    else:
        xfa, bt = _marshal(X, filters, bias)
    if VARIANT == "fp32r":
        bt = np.ascontiguousarray(bt[:FOUT])
    in_maps = [{"xf": xfa[k], "bt": bt} for k in range(NCORES)]
    res = run_bass_kernel_spmd(nc, in_maps, core_ids=list(range(NCORES)))
    LAST_RESULT = res
    outs = [res.results[k]["out"] for k in range(NCORES)]
    if VARIANT == "bd":
        return _assemble_bd(outs)
    return _assemble_r(outs) if VARIANT == "fp32r" else _assemble(outs)



# revision 37
# speedup vs baseline: 1.0424x; 1.0424x over previous
"""Locally-connected conv (BioConvolution) Trainium2 kernel.

Problem: Z[n,p,o] = relu(sum_{ijc} patch[n,p,i,j,c] * filt[p,i,j,c,o] + bias[o])
  X: (32,128,128,32) f32, filters: (1024,4,4,32,32) f32, bias: (32,)
  out: (32,32,32,32) f32.   FH=FW=4 non-overlapping patches, P=1024.

Sharding: patch-parallel over P across 8 cores. Core k owns patches
[128k,128k+128) == image rows [16k,16k+16). Each core touches only its own
X rows and filters: 16.8 MB in + 0.5 MB out per core — the true memory
roofline (~48 us at 358 GB/s per-core HBM; no operand is reused anywhere).

Host-side marshaling (part of sharding): the contraction axis must sit on
SBUF partitions for the PE, so X is pre-arranged per-core into
  xt[r, p, q, b] = X[b, 16k+4*pr+q, 4*pc+j, c]   (r = j*32+c, p = pr*32+pc)
and the filters into the matching ft[r, p, q, o]; both are packed into one
r-major array xf (data cols 0:32, filter cols 32:64) so every HBM->SBUF
DMA moves 128 partitions x multi-KB contiguous runs at line rate.

Device kernel (identical SPMD program on 8 cores), shipped variant fp32r:
  - All input loads issue from the sync engine's single HWDGE FIFO:
    strictly in-order chunk completions (concurrently-armed queues would
    round-robin and synchronize their completions, starving the PE), with
    a graduated [2,2,4] head so the first matmul starts early and a [4,4]
    tail to shorten the final dependency chain. bufs=8 double-buffering.
  - Per patch: 4 accumulating float32r matmuls (K=128, M=32 fout, N=32
    batch) — single-pass fp32 (~tf32 precision, rel err ~1.5e-4, half the
    PE instruction stream of true fp32 which lowers to LO/HI pairs).
    fp32r requires PSUM base partition 0, so 8 patches pack side-by-side
    along the free axis of one PSUM bank [32, 8x32].
  - ScalarE applies bias+ReLU per PSUM bank into an SBUF staging buffer;
    output stores ride ScalarE's own HWDGE ring LAGGED two groups behind
    the ACT stream, so their dependencies are long complete and they can
    never head-of-line block either the load FIFO or the ACT stream
    (gpsimd/SWDGE stores were tried and added multi-us Q7 drain jitter).
  - Two 4-patch mini-groups at the end halve the final
    load->matmul->ACT->store dependency chain.
Measured: ~62-66 us NEFF exec across runs (~±2 us device jitter), vs a
~48 us pure-traffic roofline at the 358 GB/s per-core HBM wall; ~8.7 us
is fixed engine-boot/Tile-preamble before the first DMA packet can flow,
~4 us is the unavoidable tail (final chain + store completion + Tile
drain barrier).
"""

import numpy as np

N, H, W, C = 32, 128, 128, 32
FH = FW = 4
FOUT = 32
NCORES = 8
PL = 128          # patches per core
NQ = 4            # K-chunks per patch (512 / 128)
KR = 128          # contraction rows per chunk (SBUF partitions)
NG = PL // 4      # 4-patch groups per core

_CACHE = {}


def _build_module(bufs=6, out_splits=8, mm_dtype="float32"):
    from concourse import bacc, tile, mybir

    nc = bacc.Bacc("TRN2", target_bir_lowering=False, debug=False, enable_asserts=False)
    dt = mybir.dt.float32
    mdt = getattr(mybir.dt, mm_dtype)
    # xf packs data and filters: [..., 0:32] = batch cols, [..., 32:64] = fout
    xf = nc.dram_tensor("xf", [KR, PL, NQ, N + FOUT], mdt, kind="ExternalInput").ap()
    bt = nc.dram_tensor("bt", [KR, 1], dt, kind="ExternalInput").ap()
    out = nc.dram_tensor("out", [KR, NG, N], dt, kind="ExternalOutput").ap()

    # Graduated chunk sizes (in patches): small first chunks so the first
    # matmul isn't gated on a full-size load sharing bandwidth round-robin.
    sizes = [2, 2, 4]
    rest = PL - sum(sizes)
    sizes += [8] * (rest // 8)
    assert sum(sizes) == PL
    GSPLIT = NG // out_splits
    relu = mybir.ActivationFunctionType.Relu

    with tile.TileContext(nc) as tc:
        with (
            tc.tile_pool(name="xfpool", bufs=bufs) as xfpool,
            tc.tile_pool(name="psum", bufs=8, space="PSUM") as psum,
            tc.tile_pool(name="misc", bufs=1) as misc,
        ):
            bias_t = misc.tile([KR, 1], dt)
            nc.sync.dma_start(bias_t[:], bt[:])
            staging = misc.tile([KR, NG, N], dt)

            p0 = 0
            for ch, PC in enumerate(sizes):
                xtile = xfpool.tile([KR, PC, NQ, N + FOUT], mdt, tag="xf")
                sl = slice(p0, p0 + PC)
                eng = nc.sync if ch % 2 == 0 else nc.scalar
                eng.dma_start(xtile[:], xf[:, sl, :, :])
                for g in range(PC // 2):
                    gg = (p0 + g * 2) // 4       # psum group id (2 patches/iter)
                    half = (p0 + g * 2) % 4      # 0 or 2: which half of the group
                    if half == 0:
                        ptile = psum.tile([KR, N], dt, tag="ps")
                    for s2 in range(2):
                        s = half + s2
                        p = g * 2 + s2
                        for q in range(NQ):
                            nc.tensor.matmul(
                                ptile[32 * s : 32 * s + 32, :],
                                xtile[:, p, q, N : N + FOUT],  # lhsT [128,32(o)]
                                xtile[:, p, q, 0:N],           # rhs  [128,32(b)]
                                start=(q == 0),
                                stop=(q == NQ - 1),
                                tile_position=(0, 32 * s),
                            )
                    if half == 2:
                        nc.scalar.activation(
                            staging[:, gg, :], ptile[:], relu, bias=bias_t[:]
                        )
                        if (gg + 1) % GSPLIT == 0:
                            osl = slice(gg + 1 - GSPLIT, gg + 1)
                            oeng = nc.sync if gg + 1 == NG else nc.gpsimd
                            oeng.dma_start(out[:, osl, :], staging[:, osl, :])
                p0 += PC
    nc.compile()
    return nc


def _build_module_r(bufs=8):
    """float32r variant: single-pass fp32 matmuls (tf32-ish precision),
    PSUM packing along the free axis (8 patches per bank) since fp32r
    requires dst base partition 0. Half the PE instruction stream of the
    fp32 variant -> fewer IRAM paging stalls."""
    from concourse import bacc, tile, mybir

    nc = bacc.Bacc("TRN2", target_bir_lowering=False, debug=False, enable_asserts=False)
    dt = mybir.dt.float32
    mdt = mybir.dt.float32r
    SG = 8                      # patches per PSUM super-group
    NSG = PL // SG              # 16
    xf = nc.dram_tensor("xf", [KR, PL, NQ, N + FOUT], mdt, kind="ExternalInput").ap()
    bt = nc.dram_tensor("bt", [FOUT, 1], dt, kind="ExternalInput").ap()
    out = nc.dram_tensor("out", [FOUT, PL, N], dt, kind="ExternalOutput").ap()

    # Graduated [2,2,4] head (earliest first matmul; measured tightest
    # variance) and a [4,4] tail that halves the final
    # load->matmul->ACT->store chain.
    sizes = [2, 2, 4] + [8] * ((PL - 16) // 8) + [4, 2, 2]
    assert sum(sizes) == PL
    # PSUM eviction groups: 8-patch banks, except two 4-patch mini-groups
    # at the end so the last matmul->ACT->store chain is half as long.
    groups = [(g * SG, SG) for g in range(NSG - 1)] + [(PL - 8, 4), (PL - 4, 4)]
    gof = {}
    for gi, (s0, gsz) in enumerate(groups):
        for i in range(gsz):
            gof[s0 + i] = (gi, i)
    relu = mybir.ActivationFunctionType.Relu

    with tile.TileContext(nc) as tc:
        with (
            tc.tile_pool(name="xfpool", bufs=bufs) as xfpool,
            tc.tile_pool(name="psum", bufs=6, space="PSUM") as psum,
            tc.tile_pool(name="misc", bufs=1) as misc,
        ):
            # bias rides the scalar ring so it doesn't burn sync's first
            # DMA slot (~0.7 us of stream start).
            bias_t = misc.tile([FOUT, 1], dt)
            nc.scalar.dma_start(bias_t[:], bt[:])
            staging = misc.tile([FOUT, PL, N], dt)

            p0 = 0
            ptile = None
            for ch, PC in enumerate(sizes):
                xtile = xfpool.tile([KR, PC, NQ, N + FOUT], mdt, tag="xf")
                # All loads on sync's single HWDGE FIFO: strictly in-order
                # completions. (Arming chunk 0 on the scalar ring was tried
                # and is bimodal: when sync's big queue gets ahead, chunk 0
                # drains at round-robin half-rate and the in-order PE
                # consumption slips ~8 us.)
                nc.sync.dma_start(xtile[:], xf[:, p0 : p0 + PC, :, :])
                for pl in range(PC):
                    p = p0 + pl
                    gi, i = gof[p]
                    s0, gsz = groups[gi]
                    if i == 0:
                        ptile = psum.tile([FOUT, SG, N], dt, tag="ps")
                    for q in range(NQ):
                        nc.tensor.matmul(
                            ptile[:, i, :],
                            xtile[:, pl, q, N : N + FOUT],  # lhsT [128,32(o)]
                            xtile[:, pl, q, 0:N],           # rhs  [128,32(b)]
                            start=(q == 0),
                            stop=(q == NQ - 1),
                        )
                    if i == gsz - 1:
                        nc.scalar.activation(
                            staging[:, s0 : s0 + gsz, :],
                            ptile[:, :gsz, :],
                            relu,
                            bias=bias_t[:],
                        )
                        # Stores also ride the scalar ring, LAGGED two groups
                        # behind the ACT stream: their ACT dependency is long
                        # complete, so they never stall scalar (and the sync
                        # load ring is untouched). The final two stores are
                        # pure program-order after the last ACT.
                        if gi == len(groups) - 1:
                            a = groups[gi - 2][0]
                            nc.scalar.dma_start(
                                out[:, a:s0, :], staging[:, a:s0, :]
                            )
                            nc.scalar.dma_start(
                                out[:, s0:PL, :], staging[:, s0:PL, :]
                            )
                        elif gi % 2 == 1 and gi >= 3:
                            a = groups[gi - 3][0]
                            b = groups[gi - 1][0]
                            nc.scalar.dma_start(
                                out[:, a:b, :], staging[:, a:b, :]
                            )
                p0 += PC
    nc.compile()
    return nc


def _build_module_bf16(bufs=6):
    """bf16 variant: inputs cast to bf16 host-side (halves HBM traffic, the
    true bottleneck; fp32 PSUM accumulation keeps rel err ~1e-3). Matmuls
    are column-tiled: 4 patches' [128,32] filter blocks sit on the 4
    column strips of the PE array via tile_position=(0,32s), emitted
    q-outer so the 4 strips run concurrently (per-subarray concurrency)
    and the PE stream can't pace the now-faster load stream."""
    from concourse import bacc, tile, mybir

    nc = bacc.Bacc("TRN2", target_bir_lowering=False, debug=False, enable_asserts=False)
    dt = mybir.dt.float32
    mdt = mybir.dt.bfloat16
    xf = nc.dram_tensor("xf", [KR, PL, NQ, N + FOUT], mdt, kind="ExternalInput").ap()
    bt = nc.dram_tensor("bt", [KR, 1], dt, kind="ExternalInput").ap()
    # Output stored bf16 (host upcasts): halves store traffic and staging.
    out = nc.dram_tensor("out", [KR, NG, N], mdt, kind="ExternalOutput").ap()

    # Graduated head (earliest first matmul) and a short tail that
    # shortens the final load->matmul->ACT->store chain. 16-patch main
    # chunks keep per-arm transfer (~1.4 us) above the sync engine's
    # ~0.65 us arm rate and halve descriptor volume vs 8.
    sizes = [2, 2, 4, 8] + [16] * 6 + [8, 4, 2, 2]
    assert sum(sizes) == PL
    # ACT groups: 8 patches (two 4-patch col-tile passes side by side on
    # the PSUM free axis), except two 4-patch mini-groups at the end.
    groups = [(g * 8, 8) for g in range((PL - 8) // 8)] + [(PL - 8, 4), (PL - 4, 4)]
    gof = {}
    for gi, (s0, gsz) in enumerate(groups):
        for i in range(gsz):
            gof[s0 + i] = gi
    relu = mybir.ActivationFunctionType.Relu

    with tile.TileContext(nc) as tc:
        with (
            # The whole per-core input (64 KB/partition) fits in SBUF, so
            # every chunk gets its own buffer: load arms carry no
            # buffer-recycle waits and all issue back-to-back at the start.
            tc.tile_pool(name="xfpool", bufs=len(sizes)) as xfpool,
            # One PSUM tile per ACT group (17 x 256B/partition): no PSUM
            # recycling, so matmul groups never stall on an ACT 8 groups
            # back (the recycle coupling cost multi-us hiccups).
            tc.tile_pool(name="psum", bufs=17, space="PSUM") as psum,
            tc.tile_pool(name="misc", bufs=1) as misc,
        ):
            bias_t = misc.tile([KR, 1], dt)
            nc.scalar.dma_start(bias_t[:], bt[:])
            staging = misc.tile([KR, NG, N], mdt)

            # All load arms first, in program order: any completion-sem
            # reuse then only ever waits on an earlier LOAD (long done) —
            # never on a store, whose packets drain at a trickle behind
            # the load queue and would stall the whole load stream.
            xtiles = []
            p0 = 0
            for PC in sizes:
                xtile = xfpool.tile([KR, PC, NQ, N + FOUT], mdt, tag="xf")
                nc.sync.dma_start(xtile[:], xf[:, p0 : p0 + PC, :, :])
                xtiles.append((p0, PC, xtile))
                p0 += PC

            ptile = None
            for p0, PC, xtile in xtiles:
                # Matmuls go q-outer within each 4-patch col-tile pass so
                # consecutive instructions hit different column strips.
                for b0 in range(0, PC, 4):
                    npass = min(4, PC - b0)        # patches in this pass
                    p = p0 + b0
                    gi = gof[p]
                    s0, gsz = groups[gi]
                    j = (p - s0) // 4              # free-axis slot in psum tile
                    if p == s0:
                        ptile = psum.tile([KR, gsz // 4, N], dt, tag="ps")
                    for q in range(NQ):
                        for si in range(npass):
                            s = (p + si - s0) % 4
                            nc.tensor.matmul(
                                ptile[32 * s : 32 * s + 32, j, :],
                                xtile[:, b0 + si, q, N : N + FOUT],
                                xtile[:, b0 + si, q, 0:N],
                                start=(q == 0),
                                stop=(q == NQ - 1),
                                tile_position=(0, 32 * s),
                            )
                    if p + npass == s0 + gsz:
                        g4 = s0 // 4
                        ng4 = gsz // 4
                        nc.scalar.activation(
                            staging[:, g4 : g4 + ng4, :],
                            ptile[:, :ng4, :],
                            relu,
                            bias=bias_t[:],
                        )
                        # Two stores at the end, on sync's queue behind the
                        # loads (in-order: packets flow the moment the load
                        # stream drains). Arm the bulk store a few groups
                        # early so its descriptor fetch overlaps the
                        # remaining loads; the last arm covers only the
                        # final 4 groups to keep the tail chain short.
                        if gi == len(groups) - 4:
                            a = g4 + ng4
                            nc.sync.dma_start(
                                out[:, 0:a, :], staging[:, 0:a, :]
                            )
                        elif gi == len(groups) - 1:
                            a = groups[len(groups) - 4][0] // 4 + 2
                            e = PL // 4
                            nc.sync.dma_start(
                                out[:, a:e, :], staging[:, a:e, :]
                            )
    nc.compile()
    return nc


def _build_module_ct2():
    """Shared-LDWEIGHTS col-tiled variant. The tensor engine's sustained
    issue rate is ~34 ns/instruction (NX fetch-limited; bursts only drain
    the 64-deep queue), so the 1024-instruction LDW+MM stream of the
    per-patch variant paces the whole kernel. Here ONE 128-column
    LDWEIGHTS per (4-patch group, q) loads all 4 strips' filters and the
    4 matmuls are emitted non-self-loading (InstMatmult.ldweights=False):
    640 tensor instructions. A ~3.4 us warm-up primer of dummy matmuls at
    program start flips the PE's HAM clock gate to 2.4 GHz before the
    real stream begins."""
    from concourse import bacc, tile, mybir

    nc = bacc.Bacc("TRN2", target_bir_lowering=False, debug=False, enable_asserts=False)
    dt = mybir.dt.float32
    mdt = mybir.dt.bfloat16
    xf = nc.dram_tensor("xf", [KR, PL, NQ, N + FOUT], mdt, kind="ExternalInput").ap()
    bt = nc.dram_tensor("bt", [KR, 1], dt, kind="ExternalInput").ap()
    out = nc.dram_tensor("out", [KR, NG, N], mdt, kind="ExternalOutput").ap()

    sizes = [2, 2, 4, 8] + [16] * 6 + [8, 4, 2, 2]
    assert sum(sizes) == PL
    groups = [(g * 8, 8) for g in range((PL - 8) // 8)] + [(PL - 8, 4), (PL - 4, 4)]
    gof = {}
    for gi, (s0, gsz) in enumerate(groups):
        for i in range(gsz):
            gof[s0 + i] = gi
    relu = mybir.ActivationFunctionType.Relu

    with tile.TileContext(nc) as tc:
        with (
            tc.tile_pool(name="xfpool", bufs=len(sizes)) as xfpool,
            tc.tile_pool(name="psum", bufs=7, space="PSUM") as psum,
            tc.tile_pool(name="pscr", bufs=1, space="PSUM") as pscr,
            tc.tile_pool(name="misc", bufs=1) as misc,
        ):
            bias_t = misc.tile([KR, 1], dt)
            nc.scalar.dma_start(bias_t[:], bt[:])
            staging = misc.tile([KR, NG, N], mdt)

            # HAM warm-up primer: ~9 zero x zero matmuls keep the PE array
            # busy from program start (~6.7 us) until real data arrives
            # (~10.3 us) so the clock gate is at 8/8 for the whole stream.
            scratch = misc.tile([KR, 640], mdt)
            nc.gpsimd.memset(scratch[:], 0)
            pdummy = pscr.tile([KR, 512], dt)
            for _ in range(9):
                nc.tensor.matmul(
                    pdummy[:],
                    scratch[:, 512:640],
                    scratch[:, 0:512],
                    start=True,
                    stop=True,
                    skip_group_check=True,
                )

            xtiles = []
            p0 = 0
            for PC in sizes:
                xtile = xfpool.tile([KR, PC, NQ, N + FOUT], mdt, tag="xf")
                nc.sync.dma_start(xtile[:], xf[:, p0 : p0 + PC, :, :])
                xtiles.append((p0, PC, xtile))
                p0 += PC

            ptile = None
            for p0, PC, xtile in xtiles:
                for b0 in range(0, PC, 4):
                    npass = min(4, PC - b0)
                    p = p0 + b0
                    gi = gof[p]
                    s0, gsz = groups[gi]
                    j = (p - s0) // 4
                    if p == s0:
                        ptile = psum.tile([KR, gsz // 4, N], dt, tag="ps")
                    s_start = (p - s0) % 4
                    for q in range(NQ):
                        # One LDW covers this pass's patches across the
                        # column strips (columns 32*s_start + p_local*32+o).
                        nc.tensor.ldweights(
                            xtile[:, b0 : b0 + npass, q, N : N + FOUT],
                            tile_position=(0, 32 * s_start),
                        )
                        for si in range(npass):
                            s = (p + si - s0) % 4
                            mm = nc.tensor.matmul(
                                ptile[32 * s : 32 * s + 32, j, :],
                                xtile[:, b0 + si, q, N : N + FOUT],
                                xtile[:, b0 + si, q, 0:N],
                                start=(q == 0),
                                stop=(q == NQ - 1),
                                tile_position=(0, 32 * s),
                                skip_group_check=True,
                            )
                            mm.ins.ldweights = False
                    if p + npass == s0 + gsz:
                        g4 = s0 // 4
                        ng4 = gsz // 4
                        nc.scalar.activation(
                            staging[:, g4 : g4 + ng4, :],
                            ptile[:, :ng4, :],
                            relu,
                            bias=bias_t[:],
                        )
                        if gi == len(groups) - 4:
                            a = g4 + ng4
                            nc.sync.dma_start(
                                out[:, 0:a, :], staging[:, 0:a, :]
                            )
                        elif gi == len(groups) - 1:
                            a = groups[len(groups) - 4][0] // 4 + 2
                            e = PL // 4
                            nc.sync.dma_start(
                                out[:, a:e, :], staging[:, a:e, :]
                            )
    nc.compile()
    return nc


def _build_module_bd2():
    """2-patch-block variant. The tensor engine's sustained issue rate is
    ~34 ns/instruction (NX fetch-limited), so the 1024-instruction
    per-patch stream (512 self-loading matmuls) paces the kernel at
    ~35 us. Here each matmul covers TWO patches: stationary = filters of
    2 patches [128, (p,o)=64], moving = X of 2 patches [128, (p',b)=64],
    out = [64, 64] of which the two diagonal 32x32 blocks are wanted Z.
    Two blocks per 4-patch group sit at column positions 0/64 and overlap
    in the array. 256 matmuls = 512 tensor fetches. Bias+ReLU runs in the
    diagonal extraction: scalar ACT (bias AP + Relu) for the lower half,
    DVE tensor_scalar(add bias, max 0) for the upper half."""
    from concourse import bacc, tile, mybir

    nc = bacc.Bacc("TRN2", target_bir_lowering=False, debug=False, enable_asserts=False)
    dt = mybir.dt.float32
    mdt = mybir.dt.bfloat16
    NB = PL // 2                       # 2-patch blocks per core
    # xf cols per (block, q): [0:64] = X of the 2 patches ((w,b) flat),
    # [64:128] = filters ((w,o) flat) — contiguous, single-free-dim APs.
    xf = nc.dram_tensor("xf", [KR, NB, NQ, 128], mdt, kind="ExternalInput").ap()
    bt = nc.dram_tensor("bt", [KR, 1], dt, kind="ExternalInput").ap()
    out = nc.dram_tensor("out", [KR, NG, N], mdt, kind="ExternalOutput").ap()

    sizes = [1, 1, 2, 4] + [8] * 6 + [4, 2, 2]      # in blocks
    assert sum(sizes) == NB
    # PSUM groups: 8 groups of 8 blocks = 8 tiles = all 8 banks, so the
    # pool NEVER recycles a bank. This is load-bearing for correctness:
    # Tile's pool-recycle waits only track the scalar reader of a PSUM
    # tile (verified in the IR), so any recycled bank's matmuls could
    # race the vector extraction.
    relu = mybir.ActivationFunctionType.Relu
    add_op = mybir.AluOpType.add
    max_op = mybir.AluOpType.max

    with tile.TileContext(nc) as tc:
        with (
            tc.tile_pool(name="xfpool", bufs=len(sizes)) as xfpool,
            # ONE psum pool shared by both extraction engines. Tile chains
            # cross-engine readers of a pooled tile (serializing vector
            # extractions ~300ns behind scalar's) — that chain is what
            # makes the pool-recycle waits sound: with per-engine pools the
            # recycled banks' matmuls carried NO wait on the DVE reader
            # (verified in the IR) and intermittently raced it.
            tc.tile_pool(name="psum", bufs=8, space="PSUM") as psum,
            tc.tile_pool(name="misc", bufs=1) as misc,
        ):
            bias_t = misc.tile([KR, 1], dt)
            nc.scalar.dma_start(bias_t[:], bt[:])
            staging = misc.tile([KR, NG, N], mdt)

            xtiles = []
            b0 = 0
            for BC in sizes:
                xtile = xfpool.tile([KR, BC, NQ, 128], mdt, tag="xf")
                nc.sync.dma_start(xtile[:], xf[:, b0 : b0 + BC, :, :])
                xtiles.append((b0, BC, xtile))
                b0 += BC

            ptile = None

            def extract(gi, ptile, j_lo, j_hi):
                # Diagonal extraction + bias + ReLU for psum slots
                # [j_lo, j_hi) of group gi; scalar takes the lower block
                # half, DVE the upper.
                for h in range(2):
                    for w in range(2):
                        pl4 = 2 * h + w
                        pr = 64 * h + 32 * w
                        src = ptile[pr : pr + 32, j_lo:j_hi, 32 * w : 32 * w + 32]
                        dst = staging[
                            32 * pl4 : 32 * pl4 + 32,
                            4 * gi + j_lo : 4 * gi + j_hi,
                            :,
                        ]
                        bsl = bias_t[pr : pr + 32]
                        if h == 0:
                            nc.scalar.activation(dst, src, relu, bias=bsl)
                        else:
                            nc.vector.tensor_scalar(
                                dst, src, bsl, 0.0, add_op, max_op
                            )

            for c0, BC, xtile in xtiles:
                for j0 in range(0, BC, 2):
                    npair = min(2, BC - j0)
                    blk0 = c0 + j0
                    gi = blk0 // 8                 # psum group (bank)
                    j = (blk0 % 8) // 2            # psum free slot (pair idx)
                    if blk0 % 8 == 0:
                        ptile = psum.tile([KR, 4, 64], dt, tag="ps")
                    # q-outer, block-inner: the pair's matmuls overlap in
                    # the array (column positions 0/64) so the chain is
                    # ~half as long as two serial per-block chains.
                    for q in range(NQ):
                        for h in range(npair):
                            blk = blk0 + h
                            off = 64 * (blk % 2)   # block position: 0 or 64
                            nc.tensor.matmul(
                                ptile[off : off + 64, j, :],
                                xtile[:, j0 + h, q, 64:128],  # stationary: filters
                                xtile[:, j0 + h, q, 0:64],    # moving: X
                                start=(q == 0),
                                stop=(q == NQ - 1),
                                tile_position=(0, off),
                                skip_group_check=True,
                            )
                    blk = blk0 + npair - 1
                    # Single-stage extraction at group end: no tensor write
                    # ever follows a read of the tile, so Tile emits no
                    # (scalar-only, conservative) tile-WAR waits that would
                    # couple the PE to the scalar ACT stream.
                    if blk % 8 == 7:
                        extract(gi, ptile, 0, 4)
                        if gi == 6:
                            nc.sync.dma_start(
                                out[:, 0:28, :], staging[:, 0:28, :]
                            )
                        elif gi == 7:
                            # Final store on the scalar ring: its queue is
                            # empty, so descriptor fetch and packets
                            # overlap the bulk store draining behind Q1.
                            nc.scalar.dma_start(
                                out[:, 28:32, :], staging[:, 28:32, :]
                            )
    nc.compile()
    return nc


def _build_module_bd():
    """Block-diagonal variant: per 4-patch group and K-chunk q, ONE
    128-column LDWEIGHTS loads the 4 patches' X [128, (p,b)=128] as the
    stationary operand and ONE matmul streams the 4 patches' filters
    [128, (p',o)=128] as the moving operand, computing all 16 cross
    blocks out[(p,b),(p',o)] of which the 4 diagonal p==p' blocks are the
    wanted Z. 8 tensor instructions per group (~280 total) instead of 32
    (~1024): the tensor stream was issue-limited at ~34 ns/instruction,
    not FLOP-limited, and 4x fewer instructions takes it off the critical
    path. Bias is folded in as a K=1 rank-one matmul (ones x bias) so the
    diagonal extraction is a pure ReLU, split across the scalar AND
    vector engines (2 strips each)."""
    from concourse import bacc, tile, mybir

    nc = bacc.Bacc("TRN2", target_bir_lowering=False, debug=False, enable_asserts=False)
    dt = mybir.dt.float32
    mdt = mybir.dt.bfloat16
    NG4 = PL // 4                      # 4-patch groups per core
    # xf2 cols: [0:128] = X block (p_local*32+b), [128:256] = filter block
    # (p_local*32+o), both contiguous so LDWEIGHTS gets FWL and the moving
    # stream is a single run.
    xf = nc.dram_tensor("xf", [KR, NG4, NQ, 256], mdt, kind="ExternalInput").ap()
    # bt: [0:128] = ones (K=1 stationary for the bias matmul),
    # [128:640] = bias tiled 16x (moving operand, 128 per group slot).
    bt = nc.dram_tensor("bt", [1, 640], mdt, kind="ExternalInput").ap()
    out = nc.dram_tensor("out", [KR, NG4, FOUT], dt, kind="ExternalOutput").ap()

    # Chunk sizes in 4-patch groups; short head and tail chunks keep the
    # first matmul early and the final load->mm->relu->store chain short.
    sizes = [1, 1, 2] + [4] * 6 + [2, 1, 1]
    assert sum(sizes) == NG4
    relu = mybir.ActivationFunctionType.Relu

    with tile.TileContext(nc) as tc:
        with (
            tc.tile_pool(name="xfpool", bufs=len(sizes)) as xfpool,
            tc.tile_pool(name="psum", bufs=4, space="PSUM") as psum,
            tc.tile_pool(name="misc", bufs=1) as misc,
        ):
            bias_t = misc.tile([1, 640], mdt)
            nc.scalar.dma_start(bias_t[:], bt[:])
            staging = misc.tile([KR, NG4, FOUT], dt)

            # All load arms first (whole input resident in SBUF): no
            # buffer-recycle waits, and completion-sem reuse only ever
            # references an earlier load.
            xtiles = []
            g0 = 0
            for GC in sizes:
                xtile = xfpool.tile([KR, GC, NQ, 256], mdt, tag="xf")
                nc.sync.dma_start(xtile[:], xf[:, g0 : g0 + GC, :, :])
                xtiles.append((g0, GC, xtile))
                g0 += GC

            for ci, (g0, GC, xtile) in enumerate(xtiles):
                ptile = psum.tile([KR, GC, 128], dt, tag="ps")
                for j in range(GC):
                    for q in range(NQ):
                        nc.tensor.matmul(
                            ptile[:, j, :],
                            xtile[:, j, q, 0:128],      # stationary: X
                            xtile[:, j, q, 128:256],    # moving: filters
                            start=(q == 0),
                            stop=False,
                            skip_group_check=True,
                        )
                    # K=1 matmul adds bias[o] to every column of region j.
                    # Must come before the NEXT region's start=True matmul:
                    # start clears has_written beyond its own out region,
                    # and a start=False write onto cleared bits OVERWRITES
                    # instead of accumulating.
                    nc.tensor.matmul(
                        ptile[:, j, :],
                        bias_t[:, 0:128],
                        bias_t[:, 128:256],
                        start=False,
                        stop=True,
                        skip_group_check=True,
                    )
                # Diagonal extraction + ReLU, 2 strips on scalar, 2 on DVE.
                for s in range(4):
                    src = ptile[32 * s : 32 * s + 32, :, 32 * s : 32 * s + 32]
                    dst = staging[32 * s : 32 * s + 32, g0 : g0 + GC, :]
                    if s < 2:
                        nc.scalar.activation(dst, src, relu)
                    else:
                        nc.vector.tensor_scalar_max(dst, src, 0.0)
                if ci == len(sizes) - 2:
                    a = g0 + GC
                    nc.sync.dma_start(out[:, 0:a, :], staging[:, 0:a, :])
                elif ci == len(sizes) - 1:
                    a = g0
                    nc.sync.dma_start(out[:, a:NG4, :], staging[:, a:NG4, :])
    nc.compile()
    return nc


def _get_module():
    if "nc" not in _CACHE:
        _CACHE["nc"] = _build_module()
    return _CACHE["nc"]


def _marshal(X, filters, bias, mdtype=np.float32):
    """Shard + lay out full inputs into per-core device arrays."""
    X = np.ascontiguousarray(np.asarray(X, dtype=np.float32))
    filters = np.ascontiguousarray(np.asarray(filters, dtype=np.float32))
    bias = np.asarray(bias, dtype=np.float32)

    # X: (b, core, pr, i, pc, j, c) -> (core, j, c, pr, pc, i, b)
    xv = X.reshape(N, NCORES, 4, FH, 32, FW, C)
    xt = xv.transpose(1, 5, 6, 2, 4, 3, 0).reshape(NCORES, KR, PL, NQ, N)
    # filters: (core, p, i, j, c, o) -> (core, j, c, p, i, o)
    fv = filters.reshape(NCORES, PL, FH, FW, C, FOUT)
    ft = fv.transpose(0, 3, 4, 1, 2, 5).reshape(NCORES, KR, PL, NQ, FOUT)
    xfa = np.concatenate([xt, ft], axis=4)
    if xfa.dtype != mdtype:
        xfa = xfa.astype(mdtype)  # round-to-nearest-even
    xfa = np.ascontiguousarray(xfa)
    bt = np.ascontiguousarray(np.tile(bias, 4).reshape(KR, 1))
    return xfa, bt


def _assemble(outs):
    """Per-core out [128=(s,o), NG, N] -> full (N, 32, 32, FOUT)."""
    z = np.stack(outs).astype(np.float32)               # (core, (s,o), g, b)
    z = z.reshape(NCORES, 4, FOUT, NG, N)               # (core, s, o, g, b)
    z = z.transpose(4, 0, 3, 1, 2)                      # (b, core, g, s, o)
    z = z.reshape(N, NCORES, PL, FOUT)                  # p_loc = 4*g + s
    z = z.reshape(N, NCORES * 4, 32, FOUT)              # (b, pr_glob, pc, o)
    return np.ascontiguousarray(z)


def _assemble_r(outs):
    """Per-core out [FOUT, PL, N] -> full (N, 32, 32, FOUT)."""
    z = np.stack(outs)                                  # (core, o, p, b)
    z = z.transpose(3, 0, 2, 1)                         # (b, core, p, o)
    return np.ascontiguousarray(z.reshape(N, 32, 32, FOUT))


def _marshal_bd2(X, filters, bias):
    """Pair-blocked layout for bd2: xf[r, blk, q, 0:64] = X of block blk's
    2 patches ((w,b) flattened), [.., 64:128] = the matching filter block
    ((w,o) flattened); both contiguous (single-free-dim matmul APs)."""
    import ml_dtypes

    bf16 = ml_dtypes.bfloat16
    NB = PL // 2
    X = np.ascontiguousarray(np.asarray(X, dtype=np.float32))
    filters = np.ascontiguousarray(np.asarray(filters, dtype=np.float32))
    bias = np.asarray(bias, dtype=np.float32)

    xv = X.reshape(N, NCORES, 4, FH, 32, FW, C)
    xt = xv.transpose(1, 5, 6, 2, 4, 3, 0).reshape(NCORES, KR, PL, NQ, N)
    xt = xt.reshape(NCORES, KR, NB, 2, NQ, N).transpose(0, 1, 2, 4, 3, 5)
    xt = xt.reshape(NCORES, KR, NB, NQ, 64)
    fv = filters.reshape(NCORES, PL, FH, FW, C, FOUT)
    ft = fv.transpose(0, 3, 4, 1, 2, 5).reshape(NCORES, KR, PL, NQ, FOUT)
    ft = ft.reshape(NCORES, KR, NB, 2, NQ, FOUT).transpose(0, 1, 2, 4, 3, 5)
    ft = ft.reshape(NCORES, KR, NB, NQ, 64)
    xfa = np.ascontiguousarray(np.concatenate([xt, ft], axis=4).astype(bf16))
    bt = np.ascontiguousarray(np.tile(bias, 4).reshape(KR, 1))
    return xfa, bt


def _marshal_bd(X, filters, bias):
    """Block-diagonal layout: xf2[r, g, q, 0:128] = X of group g's 4
    patches ((p_local, b) flattened), [.., 128:256] = the matching filter
    block ((p_local, o) flattened); bt = [ones(128), tile(bias, 16)]."""
    import ml_dtypes

    bf16 = ml_dtypes.bfloat16
    X = np.ascontiguousarray(np.asarray(X, dtype=np.float32))
    filters = np.ascontiguousarray(np.asarray(filters, dtype=np.float32))
    bias = np.asarray(bias, dtype=np.float32)

    NG4 = PL // 4
    # X: (b, core, pr, i, pc, j, c) -> (core, j, c, pr, pc, i, b)
    xv = X.reshape(N, NCORES, 4, FH, 32, FW, C)
    xt = xv.transpose(1, 5, 6, 2, 4, 3, 0).reshape(NCORES, KR, PL, NQ, N)
    # -> (core, r, g, q, p_local, b)
    xt = xt.reshape(NCORES, KR, NG4, 4, NQ, N).transpose(0, 1, 2, 4, 3, 5)
    xt = xt.reshape(NCORES, KR, NG4, NQ, 128)
    fv = filters.reshape(NCORES, PL, FH, FW, C, FOUT)
    ft = fv.transpose(0, 3, 4, 1, 2, 5).reshape(NCORES, KR, PL, NQ, FOUT)
    ft = ft.reshape(NCORES, KR, NG4, 4, NQ, FOUT).transpose(0, 1, 2, 4, 3, 5)
    ft = ft.reshape(NCORES, KR, NG4, NQ, 128)
    xfa = np.ascontiguousarray(
        np.concatenate([xt, ft], axis=4).astype(bf16)
    )
    bt = np.ascontiguousarray(
        np.concatenate([np.ones(128, np.float32), np.tile(bias, 16)])
        .astype(bf16)
        .reshape(1, 640)
    )
    return xfa, bt


def _assemble_bd(outs):
    """Per-core out [(s,b)=128, NG4, FOUT] -> full (N, 32, 32, FOUT)."""
    NG4 = PL // 4
    z = np.stack(outs)                                  # (core, (s,b), g, o)
    z = z.reshape(NCORES, 4, N, NG4, FOUT)              # (core, s, b, g, o)
    z = z.transpose(2, 0, 3, 1, 4)                      # (b, core, g, s, o)
    z = z.reshape(N, NCORES, PL, FOUT)                  # p_loc = 4*g + s
    return np.ascontiguousarray(z.reshape(N, NCORES * 4, 32, FOUT))


LAST_RESULT = None
VARIANT = "bd2"


def kernel(X, filters, bias):
    global LAST_RESULT
    from concourse import bass_utils
    from concourse.bass_utils import run_bass_kernel_spmd

    # If tracing is enabled in the environment, keep the artifact upload
    # local so a missing bucket can't fail the run.
    bass_utils.upload_artifacts = lambda tmpdir: f"local://{tmpdir}"

    if "nc" not in _CACHE:
        _CACHE["nc"] = {
            "fp32r": _build_module_r,
            "fp32": _build_module,
            "bf16ct": _build_module_bf16,
            "ct2": _build_module_ct2,
            "bd2": _build_module_bd2,
            "bd": _build_module_bd,
        }[VARIANT]()
    nc = _CACHE["nc"]
    if VARIANT == "bd":
        xfa, bt = _marshal_bd(X, filters, bias)
    elif VARIANT == "bd2":
        xfa, bt = _marshal_bd2(X, filters, bias)
    elif VARIANT in ("bf16ct", "ct2"):
        import ml_dtypes

        xfa, bt = _marshal(X, filters, bias, mdtype=ml_dtypes.bfloat16)
    else:
        xfa, bt = _marshal(X, filters, bias)
    if VARIANT == "fp32r":
        bt = np.ascontiguousarray(bt[:FOUT])
    in_maps = [{"xf": xfa[k], "bt": bt} for k in range(NCORES)]
    res = run_bass_kernel_spmd(nc, in_maps, core_ids=list(range(NCORES)))
    LAST_RESULT = res
    outs = [res.results[k]["out"] for k in range(NCORES)]
    if VARIANT == "bd":
        return _assemble_bd(outs)
    return _assemble_r(outs) if VARIANT == "fp32r" else _assemble(outs)



# revision 39
# speedup vs baseline: 1.0809x; 1.0369x over previous
"""Locally-connected conv (BioConvolution) Trainium2 kernel.

Problem: Z[n,p,o] = relu(sum_{ijc} patch[n,p,i,j,c] * filt[p,i,j,c,o] + bias[o])
  X: (32,128,128,32) f32, filters: (1024,4,4,32,32) f32, bias: (32,)
  out: (32,32,32,32) f32.   FH=FW=4 non-overlapping patches, P=1024.

Sharding: patch-parallel over P across 8 cores. Core k owns patches
[128k,128k+128) == image rows [16k,16k+16). Each core touches only its own
X rows and filters: 16.8 MB in + 0.5 MB out per core — the true memory
roofline (~48 us at 358 GB/s per-core HBM; no operand is reused anywhere).

Host-side marshaling (part of sharding): the contraction axis must sit on
SBUF partitions for the PE, so X is pre-arranged per-core into
  xt[r, p, q, b] = X[b, 16k+4*pr+q, 4*pc+j, c]   (r = j*32+c, p = pr*32+pc)
and the filters into the matching ft[r, p, q, o]; both are packed into one
r-major array xf (data cols 0:32, filter cols 32:64) so every HBM->SBUF
DMA moves 128 partitions x multi-KB contiguous runs at line rate.

Device kernel (identical SPMD program on 8 cores), shipped variant fp32r:
  - All input loads issue from the sync engine's single HWDGE FIFO:
    strictly in-order chunk completions (concurrently-armed queues would
    round-robin and synchronize their completions, starving the PE), with
    a graduated [2,2,4] head so the first matmul starts early and a [4,4]
    tail to shorten the final dependency chain. bufs=8 double-buffering.
  - Per patch: 4 accumulating float32r matmuls (K=128, M=32 fout, N=32
    batch) — single-pass fp32 (~tf32 precision, rel err ~1.5e-4, half the
    PE instruction stream of true fp32 which lowers to LO/HI pairs).
    fp32r requires PSUM base partition 0, so 8 patches pack side-by-side
    along the free axis of one PSUM bank [32, 8x32].
  - ScalarE applies bias+ReLU per PSUM bank into an SBUF staging buffer;
    output stores ride ScalarE's own HWDGE ring LAGGED two groups behind
    the ACT stream, so their dependencies are long complete and they can
    never head-of-line block either the load FIFO or the ACT stream
    (gpsimd/SWDGE stores were tried and added multi-us Q7 drain jitter).
  - Two 4-patch mini-groups at the end halve the final
    load->matmul->ACT->store dependency chain.
Measured: ~62-66 us NEFF exec across runs (~±2 us device jitter), vs a
~48 us pure-traffic roofline at the 358 GB/s per-core HBM wall; ~8.7 us
is fixed engine-boot/Tile-preamble before the first DMA packet can flow,
~4 us is the unavoidable tail (final chain + store completion + Tile
drain barrier).
"""

import numpy as np

N, H, W, C = 32, 128, 128, 32
FH = FW = 4
FOUT = 32
NCORES = 8
PL = 128          # patches per core
NQ = 4            # K-chunks per patch (512 / 128)
KR = 128          # contraction rows per chunk (SBUF partitions)
NG = PL // 4      # 4-patch groups per core

_CACHE = {}


def _build_module(bufs=6, out_splits=8, mm_dtype="float32"):
    from concourse import bacc, tile, mybir

    nc = bacc.Bacc("TRN2", target_bir_lowering=False, debug=False, enable_asserts=False)
    dt = mybir.dt.float32
    mdt = getattr(mybir.dt, mm_dtype)
    # xf packs data and filters: [..., 0:32] = batch cols, [..., 32:64] = fout
    xf = nc.dram_tensor("xf", [KR, PL, NQ, N + FOUT], mdt, kind="ExternalInput").ap()
    bt = nc.dram_tensor("bt", [KR, 1], dt, kind="ExternalInput").ap()
    out = nc.dram_tensor("out", [KR, NG, N], dt, kind="ExternalOutput").ap()

    # Graduated chunk sizes (in patches): small first chunks so the first
    # matmul isn't gated on a full-size load sharing bandwidth round-robin.
    sizes = [2, 2, 4]
    rest = PL - sum(sizes)
    sizes += [8] * (rest // 8)
    assert sum(sizes) == PL
    GSPLIT = NG // out_splits
    relu = mybir.ActivationFunctionType.Relu

    with tile.TileContext(nc) as tc:
        with (
            tc.tile_pool(name="xfpool", bufs=bufs) as xfpool,
            tc.tile_pool(name="psum", bufs=8, space="PSUM") as psum,
            tc.tile_pool(name="misc", bufs=1) as misc,
        ):
            bias_t = misc.tile([KR, 1], dt)
            nc.sync.dma_start(bias_t[:], bt[:])
            staging = misc.tile([KR, NG, N], dt)

            p0 = 0
            for ch, PC in enumerate(sizes):
                xtile = xfpool.tile([KR, PC, NQ, N + FOUT], mdt, tag="xf")
                sl = slice(p0, p0 + PC)
                eng = nc.sync if ch % 2 == 0 else nc.scalar
                eng.dma_start(xtile[:], xf[:, sl, :, :])
                for g in range(PC // 2):
                    gg = (p0 + g * 2) // 4       # psum group id (2 patches/iter)
                    half = (p0 + g * 2) % 4      # 0 or 2: which half of the group
                    if half == 0:
                        ptile = psum.tile([KR, N], dt, tag="ps")
                    for s2 in range(2):
                        s = half + s2
                        p = g * 2 + s2
                        for q in range(NQ):
                            nc.tensor.matmul(
                                ptile[32 * s : 32 * s + 32, :],
                                xtile[:, p, q, N : N + FOUT],  # lhsT [128,32(o)]
                                xtile[:, p, q, 0:N],           # rhs  [128,32(b)]
                                start=(q == 0),
                                stop=(q == NQ - 1),
                                tile_position=(0, 32 * s),
                            )
                    if half == 2:
                        nc.scalar.activation(
                            staging[:, gg, :], ptile[:], relu, bias=bias_t[:]
                        )
                        if (gg + 1) % GSPLIT == 0:
                            osl = slice(gg + 1 - GSPLIT, gg + 1)
                            oeng = nc.sync if gg + 1 == NG else nc.gpsimd
                            oeng.dma_start(out[:, osl, :], staging[:, osl, :])
                p0 += PC
    nc.compile()
    return nc


def _build_module_r(bufs=8):
    """float32r variant: single-pass fp32 matmuls (tf32-ish precision),
    PSUM packing along the free axis (8 patches per bank) since fp32r
    requires dst base partition 0. Half the PE instruction stream of the
    fp32 variant -> fewer IRAM paging stalls."""
    from concourse import bacc, tile, mybir

    nc = bacc.Bacc("TRN2", target_bir_lowering=False, debug=False, enable_asserts=False)
    dt = mybir.dt.float32
    mdt = mybir.dt.float32r
    SG = 8                      # patches per PSUM super-group
    NSG = PL // SG              # 16
    xf = nc.dram_tensor("xf", [KR, PL, NQ, N + FOUT], mdt, kind="ExternalInput").ap()
    bt = nc.dram_tensor("bt", [FOUT, 1], dt, kind="ExternalInput").ap()
    out = nc.dram_tensor("out", [FOUT, PL, N], dt, kind="ExternalOutput").ap()

    # Graduated [2,2,4] head (earliest first matmul; measured tightest
    # variance) and a [4,4] tail that halves the final
    # load->matmul->ACT->store chain.
    sizes = [2, 2, 4] + [8] * ((PL - 16) // 8) + [4, 2, 2]
    assert sum(sizes) == PL
    # PSUM eviction groups: 8-patch banks, except two 4-patch mini-groups
    # at the end so the last matmul->ACT->store chain is half as long.
    groups = [(g * SG, SG) for g in range(NSG - 1)] + [(PL - 8, 4), (PL - 4, 4)]
    gof = {}
    for gi, (s0, gsz) in enumerate(groups):
        for i in range(gsz):
            gof[s0 + i] = (gi, i)
    relu = mybir.ActivationFunctionType.Relu

    with tile.TileContext(nc) as tc:
        with (
            tc.tile_pool(name="xfpool", bufs=bufs) as xfpool,
            tc.tile_pool(name="psum", bufs=6, space="PSUM") as psum,
            tc.tile_pool(name="misc", bufs=1) as misc,
        ):
            # bias rides the scalar ring so it doesn't burn sync's first
            # DMA slot (~0.7 us of stream start).
            bias_t = misc.tile([FOUT, 1], dt)
            nc.scalar.dma_start(bias_t[:], bt[:])
            staging = misc.tile([FOUT, PL, N], dt)

            p0 = 0
            ptile = None
            for ch, PC in enumerate(sizes):
                xtile = xfpool.tile([KR, PC, NQ, N + FOUT], mdt, tag="xf")
                # All loads on sync's single HWDGE FIFO: strictly in-order
                # completions. (Arming chunk 0 on the scalar ring was tried
                # and is bimodal: when sync's big queue gets ahead, chunk 0
                # drains at round-robin half-rate and the in-order PE
                # consumption slips ~8 us.)
                nc.sync.dma_start(xtile[:], xf[:, p0 : p0 + PC, :, :])
                for pl in range(PC):
                    p = p0 + pl
                    gi, i = gof[p]
                    s0, gsz = groups[gi]
                    if i == 0:
                        ptile = psum.tile([FOUT, SG, N], dt, tag="ps")
                    for q in range(NQ):
                        nc.tensor.matmul(
                            ptile[:, i, :],
                            xtile[:, pl, q, N : N + FOUT],  # lhsT [128,32(o)]
                            xtile[:, pl, q, 0:N],           # rhs  [128,32(b)]
                            start=(q == 0),
                            stop=(q == NQ - 1),
                        )
                    if i == gsz - 1:
                        nc.scalar.activation(
                            staging[:, s0 : s0 + gsz, :],
                            ptile[:, :gsz, :],
                            relu,
                            bias=bias_t[:],
                        )
                        # Stores also ride the scalar ring, LAGGED two groups
                        # behind the ACT stream: their ACT dependency is long
                        # complete, so they never stall scalar (and the sync
                        # load ring is untouched). The final two stores are
                        # pure program-order after the last ACT.
                        if gi == len(groups) - 1:
                            a = groups[gi - 2][0]
                            nc.scalar.dma_start(
                                out[:, a:s0, :], staging[:, a:s0, :]
                            )
                            nc.scalar.dma_start(
                                out[:, s0:PL, :], staging[:, s0:PL, :]
                            )
                        elif gi % 2 == 1 and gi >= 3:
                            a = groups[gi - 3][0]
                            b = groups[gi - 1][0]
                            nc.scalar.dma_start(
                                out[:, a:b, :], staging[:, a:b, :]
                            )
                p0 += PC
    nc.compile()
    return nc


def _build_module_bf16(bufs=6):
    """bf16 variant: inputs cast to bf16 host-side (halves HBM traffic, the
    true bottleneck; fp32 PSUM accumulation keeps rel err ~1e-3). Matmuls
    are column-tiled: 4 patches' [128,32] filter blocks sit on the 4
    column strips of the PE array via tile_position=(0,32s), emitted
    q-outer so the 4 strips run concurrently (per-subarray concurrency)
    and the PE stream can't pace the now-faster load stream."""
    from concourse import bacc, tile, mybir

    nc = bacc.Bacc("TRN2", target_bir_lowering=False, debug=False, enable_asserts=False)
    dt = mybir.dt.float32
    mdt = mybir.dt.bfloat16
    xf = nc.dram_tensor("xf", [KR, PL, NQ, N + FOUT], mdt, kind="ExternalInput").ap()
    bt = nc.dram_tensor("bt", [KR, 1], dt, kind="ExternalInput").ap()
    # Output stored bf16 (host upcasts): halves store traffic and staging.
    out = nc.dram_tensor("out", [KR, NG, N], mdt, kind="ExternalOutput").ap()

    # Graduated head (earliest first matmul) and a short tail that
    # shortens the final load->matmul->ACT->store chain. 16-patch main
    # chunks keep per-arm transfer (~1.4 us) above the sync engine's
    # ~0.65 us arm rate and halve descriptor volume vs 8.
    sizes = [2, 2, 4, 8] + [16] * 6 + [8, 4, 2, 2]
    assert sum(sizes) == PL
    # ACT groups: 8 patches (two 4-patch col-tile passes side by side on
    # the PSUM free axis), except two 4-patch mini-groups at the end.
    groups = [(g * 8, 8) for g in range((PL - 8) // 8)] + [(PL - 8, 4), (PL - 4, 4)]
    gof = {}
    for gi, (s0, gsz) in enumerate(groups):
        for i in range(gsz):
            gof[s0 + i] = gi
    relu = mybir.ActivationFunctionType.Relu

    with tile.TileContext(nc) as tc:
        with (
            # The whole per-core input (64 KB/partition) fits in SBUF, so
            # every chunk gets its own buffer: load arms carry no
            # buffer-recycle waits and all issue back-to-back at the start.
            tc.tile_pool(name="xfpool", bufs=len(sizes)) as xfpool,
            # One PSUM tile per ACT group (17 x 256B/partition): no PSUM
            # recycling, so matmul groups never stall on an ACT 8 groups
            # back (the recycle coupling cost multi-us hiccups).
            tc.tile_pool(name="psum", bufs=17, space="PSUM") as psum,
            tc.tile_pool(name="misc", bufs=1) as misc,
        ):
            bias_t = misc.tile([KR, 1], dt)
            nc.scalar.dma_start(bias_t[:], bt[:])
            staging = misc.tile([KR, NG, N], mdt)

            # All load arms first, in program order: any completion-sem
            # reuse then only ever waits on an earlier LOAD (long done) —
            # never on a store, whose packets drain at a trickle behind
            # the load queue and would stall the whole load stream.
            xtiles = []
            p0 = 0
            for PC in sizes:
                xtile = xfpool.tile([KR, PC, NQ, N + FOUT], mdt, tag="xf")
                nc.sync.dma_start(xtile[:], xf[:, p0 : p0 + PC, :, :])
                xtiles.append((p0, PC, xtile))
                p0 += PC

            ptile = None
            for p0, PC, xtile in xtiles:
                # Matmuls go q-outer within each 4-patch col-tile pass so
                # consecutive instructions hit different column strips.
                for b0 in range(0, PC, 4):
                    npass = min(4, PC - b0)        # patches in this pass
                    p = p0 + b0
                    gi = gof[p]
                    s0, gsz = groups[gi]
                    j = (p - s0) // 4              # free-axis slot in psum tile
                    if p == s0:
                        ptile = psum.tile([KR, gsz // 4, N], dt, tag="ps")
                    for q in range(NQ):
                        for si in range(npass):
                            s = (p + si - s0) % 4
                            nc.tensor.matmul(
                                ptile[32 * s : 32 * s + 32, j, :],
                                xtile[:, b0 + si, q, N : N + FOUT],
                                xtile[:, b0 + si, q, 0:N],
                                start=(q == 0),
                                stop=(q == NQ - 1),
                                tile_position=(0, 32 * s),
                            )
                    if p + npass == s0 + gsz:
                        g4 = s0 // 4
                        ng4 = gsz // 4
                        nc.scalar.activation(
                            staging[:, g4 : g4 + ng4, :],
                            ptile[:, :ng4, :],
                            relu,
                            bias=bias_t[:],
                        )
                        # Two stores at the end, on sync's queue behind the
                        # loads (in-order: packets flow the moment the load
                        # stream drains). Arm the bulk store a few groups
                        # early so its descriptor fetch overlaps the
                        # remaining loads; the last arm covers only the
                        # final 4 groups to keep the tail chain short.
                        if gi == len(groups) - 4:
                            a = g4 + ng4
                            nc.sync.dma_start(
                                out[:, 0:a, :], staging[:, 0:a, :]
                            )
                        elif gi == len(groups) - 1:
                            a = groups[len(groups) - 4][0] // 4 + 2
                            e = PL // 4
                            nc.sync.dma_start(
                                out[:, a:e, :], staging[:, a:e, :]
                            )
    nc.compile()
    return nc


def _build_module_ct2():
    """Shared-LDWEIGHTS col-tiled variant. The tensor engine's sustained
    issue rate is ~34 ns/instruction (NX fetch-limited; bursts only drain
    the 64-deep queue), so the 1024-instruction LDW+MM stream of the
    per-patch variant paces the whole kernel. Here ONE 128-column
    LDWEIGHTS per (4-patch group, q) loads all 4 strips' filters and the
    4 matmuls are emitted non-self-loading (InstMatmult.ldweights=False):
    640 tensor instructions. A ~3.4 us warm-up primer of dummy matmuls at
    program start flips the PE's HAM clock gate to 2.4 GHz before the
    real stream begins."""
    from concourse import bacc, tile, mybir

    nc = bacc.Bacc("TRN2", target_bir_lowering=False, debug=False, enable_asserts=False)
    dt = mybir.dt.float32
    mdt = mybir.dt.bfloat16
    xf = nc.dram_tensor("xf", [KR, PL, NQ, N + FOUT], mdt, kind="ExternalInput").ap()
    bt = nc.dram_tensor("bt", [KR, 1], dt, kind="ExternalInput").ap()
    out = nc.dram_tensor("out", [KR, NG, N], mdt, kind="ExternalOutput").ap()

    sizes = [2, 2, 4, 8] + [16] * 6 + [8, 4, 2, 2]
    assert sum(sizes) == PL
    groups = [(g * 8, 8) for g in range((PL - 8) // 8)] + [(PL - 8, 4), (PL - 4, 4)]
    gof = {}
    for gi, (s0, gsz) in enumerate(groups):
        for i in range(gsz):
            gof[s0 + i] = gi
    relu = mybir.ActivationFunctionType.Relu

    with tile.TileContext(nc) as tc:
        with (
            tc.tile_pool(name="xfpool", bufs=len(sizes)) as xfpool,
            tc.tile_pool(name="psum", bufs=7, space="PSUM") as psum,
            tc.tile_pool(name="pscr", bufs=1, space="PSUM") as pscr,
            tc.tile_pool(name="misc", bufs=1) as misc,
        ):
            bias_t = misc.tile([KR, 1], dt)
            nc.scalar.dma_start(bias_t[:], bt[:])
            staging = misc.tile([KR, NG, N], mdt)

            # HAM warm-up primer: ~9 zero x zero matmuls keep the PE array
            # busy from program start (~6.7 us) until real data arrives
            # (~10.3 us) so the clock gate is at 8/8 for the whole stream.
            scratch = misc.tile([KR, 640], mdt)
            nc.gpsimd.memset(scratch[:], 0)
            pdummy = pscr.tile([KR, 512], dt)
            for _ in range(9):
                nc.tensor.matmul(
                    pdummy[:],
                    scratch[:, 512:640],
                    scratch[:, 0:512],
                    start=True,
                    stop=True,
                    skip_group_check=True,
                )

            xtiles = []
            p0 = 0
            for PC in sizes:
                xtile = xfpool.tile([KR, PC, NQ, N + FOUT], mdt, tag="xf")
                nc.sync.dma_start(xtile[:], xf[:, p0 : p0 + PC, :, :])
                xtiles.append((p0, PC, xtile))
                p0 += PC

            ptile = None
            for p0, PC, xtile in xtiles:
                for b0 in range(0, PC, 4):
                    npass = min(4, PC - b0)
                    p = p0 + b0
                    gi = gof[p]
                    s0, gsz = groups[gi]
                    j = (p - s0) // 4
                    if p == s0:
                        ptile = psum.tile([KR, gsz // 4, N], dt, tag="ps")
                    s_start = (p - s0) % 4
                    for q in range(NQ):
                        # One LDW covers this pass's patches across the
                        # column strips (columns 32*s_start + p_local*32+o).
                        nc.tensor.ldweights(
                            xtile[:, b0 : b0 + npass, q, N : N + FOUT],
                            tile_position=(0, 32 * s_start),
                        )
                        for si in range(npass):
                            s = (p + si - s0) % 4
                            mm = nc.tensor.matmul(
                                ptile[32 * s : 32 * s + 32, j, :],
                                xtile[:, b0 + si, q, N : N + FOUT],
                                xtile[:, b0 + si, q, 0:N],
                                start=(q == 0),
                                stop=(q == NQ - 1),
                                tile_position=(0, 32 * s),
                                skip_group_check=True,
                            )
                            mm.ins.ldweights = False
                    if p + npass == s0 + gsz:
                        g4 = s0 // 4
                        ng4 = gsz // 4
                        nc.scalar.activation(
                            staging[:, g4 : g4 + ng4, :],
                            ptile[:, :ng4, :],
                            relu,
                            bias=bias_t[:],
                        )
                        if gi == len(groups) - 4:
                            a = g4 + ng4
                            nc.sync.dma_start(
                                out[:, 0:a, :], staging[:, 0:a, :]
                            )
                        elif gi == len(groups) - 1:
                            a = groups[len(groups) - 4][0] // 4 + 2
                            e = PL // 4
                            nc.sync.dma_start(
                                out[:, a:e, :], staging[:, a:e, :]
                            )
    nc.compile()
    return nc


def _build_module_bd2():
    """2-patch-block variant. The tensor engine's sustained issue rate is
    ~34 ns/instruction (NX fetch-limited), so the 1024-instruction
    per-patch stream (512 self-loading matmuls) paces the kernel at
    ~35 us. Here each matmul covers TWO patches: stationary = filters of
    2 patches [128, (p,o)=64], moving = X of 2 patches [128, (p',b)=64],
    out = [64, 64] of which the two diagonal 32x32 blocks are wanted Z.
    Two blocks per 4-patch group sit at column positions 0/64 and overlap
    in the array. 256 matmuls = 512 tensor fetches. Bias+ReLU runs in the
    diagonal extraction: scalar ACT (bias AP + Relu) for the lower half,
    DVE tensor_scalar(add bias, max 0) for the upper half."""
    from concourse import bacc, tile, mybir

    nc = bacc.Bacc("TRN2", target_bir_lowering=False, debug=False, enable_asserts=False)
    dt = mybir.dt.float32
    mdt = mybir.dt.bfloat16
    NB = PL // 2                       # 2-patch blocks per core
    # xf cols per (block, q): [0:64] = X of the 2 patches ((w,b) flat),
    # [64:128] = filters ((w,o) flat) — contiguous, single-free-dim APs.
    xf = nc.dram_tensor("xf", [KR, NB, NQ, 128], mdt, kind="ExternalInput").ap()
    bt = nc.dram_tensor("bt", [KR, 1], dt, kind="ExternalInput").ap()
    out = nc.dram_tensor("out", [KR, NG, N], mdt, kind="ExternalOutput").ap()

    sizes = [1, 1, 2, 4] + [8] * 6 + [4, 2, 2]      # in blocks
    assert sum(sizes) == NB
    # PSUM groups of 4 blocks (8 patches, one bank), two 2-block groups
    # at the end for a short final chain. The pool-recycle waits only
    # name the scalar reader, but the DVE reader is chained ~one
    # instruction behind scalar by Tile's cross-engine accessor ordering
    # and the recycle distance is 8 groups (~17 us of DMA) — safe margin.
    groups = [(g * 4, 4) for g in range((NB - 4) // 4)] + [(NB - 4, 2), (NB - 2, 2)]
    gof = {}
    for gi, (b0, gsz) in enumerate(groups):
        for i in range(gsz):
            gof[b0 + i] = gi
    relu = mybir.ActivationFunctionType.Relu
    add_op = mybir.AluOpType.add
    max_op = mybir.AluOpType.max

    with tile.TileContext(nc) as tc:
        with (
            tc.tile_pool(name="xfpool", bufs=len(sizes)) as xfpool,
            # ONE psum pool shared by both extraction engines. Tile chains
            # cross-engine readers of a pooled tile (serializing vector
            # extractions ~300ns behind scalar's) — that chain is what
            # makes the pool-recycle waits sound: with per-engine pools the
            # recycled banks' matmuls carried NO wait on the DVE reader
            # (verified in the IR) and intermittently raced it.
            tc.tile_pool(name="psum", bufs=8, space="PSUM") as psum,
            tc.tile_pool(name="misc", bufs=1) as misc,
        ):
            bias_t = misc.tile([KR, 1], dt)
            nc.scalar.dma_start(bias_t[:], bt[:])
            staging = misc.tile([KR, NG, N], mdt)

            xtiles = []
            b0 = 0
            for BC in sizes:
                xtile = xfpool.tile([KR, BC, NQ, 128], mdt, tag="xf")
                nc.sync.dma_start(xtile[:], xf[:, b0 : b0 + BC, :, :])
                xtiles.append((b0, BC, xtile))
                b0 += BC

            ptile = None
            for c0, BC, xtile in xtiles:
                for j0 in range(0, BC, 2):
                    npair = min(2, BC - j0)
                    blk0 = c0 + j0
                    gi = gof[blk0]
                    g0, gsz = groups[gi]
                    j = (blk0 - g0) // 2           # psum free slot (pair idx)
                    if blk0 == g0:
                        ptile = psum.tile([KR, gsz // 2, 64], dt, tag="ps")
                    # q-outer, block-inner: the pair's matmuls overlap in
                    # the array (column positions 0/64) so the chain is
                    # ~half as long as two serial per-block chains.
                    for q in range(NQ):
                        for h in range(npair):
                            blk = blk0 + h
                            off = 64 * (blk % 2)   # block position: 0 or 64
                            nc.tensor.matmul(
                                ptile[off : off + 64, j, :],
                                xtile[:, j0 + h, q, 64:128],  # stationary: filters
                                xtile[:, j0 + h, q, 0:64],    # moving: X
                                start=(q == 0),
                                stop=(q == NQ - 1),
                                tile_position=(0, off),
                                skip_group_check=True,
                            )
                    blk = blk0 + npair - 1
                    if blk == g0 + gsz - 1:
                        g4 = g0 // 2
                        ng4 = gsz // 2
                        for h in range(2):
                            for w in range(2):
                                pl4 = 2 * h + w
                                pr = 64 * h + 32 * w
                                src = ptile[pr : pr + 32, :ng4, 32 * w : 32 * w + 32]
                                dst = staging[32 * pl4 : 32 * pl4 + 32, g4 : g4 + ng4, :]
                                bsl = bias_t[pr : pr + 32]
                                if h == 0:
                                    nc.scalar.activation(dst, src, relu, bias=bsl)
                                else:
                                    nc.vector.tensor_scalar(
                                        dst, src, bsl, 0.0, add_op, max_op
                                    )
                        if gi == len(groups) - 4:
                            a = g4 + ng4
                            nc.sync.dma_start(
                                out[:, 0:a, :], staging[:, 0:a, :]
                            )
                        elif gi == len(groups) - 1:
                            # Final store on the scalar ring: its queue is
                            # empty, so descriptor fetch and packets overlap
                            # the bulk store draining behind Q1.
                            a = groups[len(groups) - 4][0] // 2 + 2
                            e = PL // 4
                            nc.scalar.dma_start(
                                out[:, a:e, :], staging[:, a:e, :]
                            )
    nc.compile()
    return nc


def _build_module_bd():
    """Block-diagonal variant: per 4-patch group and K-chunk q, ONE
    128-column LDWEIGHTS loads the 4 patches' X [128, (p,b)=128] as the
    stationary operand and ONE matmul streams the 4 patches' filters
    [128, (p',o)=128] as the moving operand, computing all 16 cross
    blocks out[(p,b),(p',o)] of which the 4 diagonal p==p' blocks are the
    wanted Z. 8 tensor instructions per group (~280 total) instead of 32
    (~1024): the tensor stream was issue-limited at ~34 ns/instruction,
    not FLOP-limited, and 4x fewer instructions takes it off the critical
    path. Bias is folded in as a K=1 rank-one matmul (ones x bias) so the
    diagonal extraction is a pure ReLU, split across the scalar AND
    vector engines (2 strips each)."""
    from concourse import bacc, tile, mybir

    nc = bacc.Bacc("TRN2", target_bir_lowering=False, debug=False, enable_asserts=False)
    dt = mybir.dt.float32
    mdt = mybir.dt.bfloat16
    NG4 = PL // 4                      # 4-patch groups per core
    # xf2 cols: [0:128] = X block (p_local*32+b), [128:256] = filter block
    # (p_local*32+o), both contiguous so LDWEIGHTS gets FWL and the moving
    # stream is a single run.
    xf = nc.dram_tensor("xf", [KR, NG4, NQ, 256], mdt, kind="ExternalInput").ap()
    # bt: [0:128] = ones (K=1 stationary for the bias matmul),
    # [128:640] = bias tiled 16x (moving operand, 128 per group slot).
    bt = nc.dram_tensor("bt", [1, 640], mdt, kind="ExternalInput").ap()
    out = nc.dram_tensor("out", [KR, NG4, FOUT], dt, kind="ExternalOutput").ap()

    # Chunk sizes in 4-patch groups; short head and tail chunks keep the
    # first matmul early and the final load->mm->relu->store chain short.
    sizes = [1, 1, 2] + [4] * 6 + [2, 1, 1]
    assert sum(sizes) == NG4
    relu = mybir.ActivationFunctionType.Relu

    with tile.TileContext(nc) as tc:
        with (
            tc.tile_pool(name="xfpool", bufs=len(sizes)) as xfpool,
            tc.tile_pool(name="psum", bufs=4, space="PSUM") as psum,
            tc.tile_pool(name="misc", bufs=1) as misc,
        ):
            bias_t = misc.tile([1, 640], mdt)
            nc.scalar.dma_start(bias_t[:], bt[:])
            staging = misc.tile([KR, NG4, FOUT], dt)

            # All load arms first (whole input resident in SBUF): no
            # buffer-recycle waits, and completion-sem reuse only ever
            # references an earlier load.
            xtiles = []
            g0 = 0
            for GC in sizes:
                xtile = xfpool.tile([KR, GC, NQ, 256], mdt, tag="xf")
                nc.sync.dma_start(xtile[:], xf[:, g0 : g0 + GC, :, :])
                xtiles.append((g0, GC, xtile))
                g0 += GC

            for ci, (g0, GC, xtile) in enumerate(xtiles):
                ptile = psum.tile([KR, GC, 128], dt, tag="ps")
                for j in range(GC):
                    for q in range(NQ):
                        nc.tensor.matmul(
                            ptile[:, j, :],
                            xtile[:, j, q, 0:128],      # stationary: X
                            xtile[:, j, q, 128:256],    # moving: filters
                            start=(q == 0),
                            stop=False,
                            skip_group_check=True,
                        )
                    # K=1 matmul adds bias[o] to every column of region j.
                    # Must come before the NEXT region's start=True matmul:
                    # start clears has_written beyond its own out region,
                    # and a start=False write onto cleared bits OVERWRITES
                    # instead of accumulating.
                    nc.tensor.matmul(
                        ptile[:, j, :],
                        bias_t[:, 0:128],
                        bias_t[:, 128:256],
                        start=False,
                        stop=True,
                        skip_group_check=True,
                    )
                # Diagonal extraction + ReLU, 2 strips on scalar, 2 on DVE.
                for s in range(4):
                    src = ptile[32 * s : 32 * s + 32, :, 32 * s : 32 * s + 32]
                    dst = staging[32 * s : 32 * s + 32, g0 : g0 + GC, :]
                    if s < 2:
                        nc.scalar.activation(dst, src, relu)
                    else:
                        nc.vector.tensor_scalar_max(dst, src, 0.0)
                if ci == len(sizes) - 2:
                    a = g0 + GC
                    nc.sync.dma_start(out[:, 0:a, :], staging[:, 0:a, :])
                elif ci == len(sizes) - 1:
                    a = g0
                    nc.sync.dma_start(out[:, a:NG4, :], staging[:, a:NG4, :])
    nc.compile()
    return nc


def _get_module():
    if "nc" not in _CACHE:
        _CACHE["nc"] = _build_module()
    return _CACHE["nc"]


def _marshal(X, filters, bias, mdtype=np.float32):
    """Shard + lay out full inputs into per-core device arrays."""
    X = np.ascontiguousarray(np.asarray(X, dtype=np.float32))
    filters = np.ascontiguousarray(np.asarray(filters, dtype=np.float32))
    bias = np.asarray(bias, dtype=np.float32)

    # X: (b, core, pr, i, pc, j, c) -> (core, j, c, pr, pc, i, b)
    xv = X.reshape(N, NCORES, 4, FH, 32, FW, C)
    xt = xv.transpose(1, 5, 6, 2, 4, 3, 0).reshape(NCORES, KR, PL, NQ, N)
    # filters: (core, p, i, j, c, o) -> (core, j, c, p, i, o)
    fv = filters.reshape(NCORES, PL, FH, FW, C, FOUT)
    ft = fv.transpose(0, 3, 4, 1, 2, 5).reshape(NCORES, KR, PL, NQ, FOUT)
    xfa = np.concatenate([xt, ft], axis=4)
    if xfa.dtype != mdtype:
        xfa = xfa.astype(mdtype)  # round-to-nearest-even
    xfa = np.ascontiguousarray(xfa)
    bt = np.ascontiguousarray(np.tile(bias, 4).reshape(KR, 1))
    return xfa, bt


def _assemble(outs):
    """Per-core out [128=(s,o), NG, N] -> full (N, 32, 32, FOUT)."""
    z = np.stack(outs).astype(np.float32)               # (core, (s,o), g, b)
    z = z.reshape(NCORES, 4, FOUT, NG, N)               # (core, s, o, g, b)
    z = z.transpose(4, 0, 3, 1, 2)                      # (b, core, g, s, o)
    z = z.reshape(N, NCORES, PL, FOUT)                  # p_loc = 4*g + s
    z = z.reshape(N, NCORES * 4, 32, FOUT)              # (b, pr_glob, pc, o)
    return np.ascontiguousarray(z)


def _assemble_r(outs):
    """Per-core out [FOUT, PL, N] -> full (N, 32, 32, FOUT)."""
    z = np.stack(outs)                                  # (core, o, p, b)
    z = z.transpose(3, 0, 2, 1)                         # (b, core, p, o)
    return np.ascontiguousarray(z.reshape(N, 32, 32, FOUT))


def _marshal_bd2(X, filters, bias):
    """Pair-blocked layout for bd2: xf[r, blk, q, 0:64] = X of block blk's
    2 patches ((w,b) flattened), [.., 64:128] = the matching filter block
    ((w,o) flattened); both contiguous (single-free-dim matmul APs)."""
    import ml_dtypes

    bf16 = ml_dtypes.bfloat16
    NB = PL // 2
    X = np.ascontiguousarray(np.asarray(X, dtype=np.float32))
    filters = np.ascontiguousarray(np.asarray(filters, dtype=np.float32))
    bias = np.asarray(bias, dtype=np.float32)

    xv = X.reshape(N, NCORES, 4, FH, 32, FW, C)
    xt = xv.transpose(1, 5, 6, 2, 4, 3, 0).reshape(NCORES, KR, PL, NQ, N)
    xt = xt.reshape(NCORES, KR, NB, 2, NQ, N).transpose(0, 1, 2, 4, 3, 5)
    xt = xt.reshape(NCORES, KR, NB, NQ, 64)
    fv = filters.reshape(NCORES, PL, FH, FW, C, FOUT)
    ft = fv.transpose(0, 3, 4, 1, 2, 5).reshape(NCORES, KR, PL, NQ, FOUT)
    ft = ft.reshape(NCORES, KR, NB, 2, NQ, FOUT).transpose(0, 1, 2, 4, 3, 5)
    ft = ft.reshape(NCORES, KR, NB, NQ, 64)
    xfa = np.ascontiguousarray(np.concatenate([xt, ft], axis=4).astype(bf16))
    bt = np.ascontiguousarray(np.tile(bias, 4).reshape(KR, 1))
    return xfa, bt


def _marshal_bd(X, filters, bias):
    """Block-diagonal layout: xf2[r, g, q, 0:128] = X of group g's 4
    patches ((p_local, b) flattened), [.., 128:256] = the matching filter
    block ((p_local, o) flattened); bt = [ones(128), tile(bias, 16)]."""
    import ml_dtypes

    bf16 = ml_dtypes.bfloat16
    X = np.ascontiguousarray(np.asarray(X, dtype=np.float32))
    filters = np.ascontiguousarray(np.asarray(filters, dtype=np.float32))
    bias = np.asarray(bias, dtype=np.float32)

    NG4 = PL // 4
    # X: (b, core, pr, i, pc, j, c) -> (core, j, c, pr, pc, i, b)
    xv = X.reshape(N, NCORES, 4, FH, 32, FW, C)
    xt = xv.transpose(1, 5, 6, 2, 4, 3, 0).reshape(NCORES, KR, PL, NQ, N)
    # -> (core, r, g, q, p_local, b)
    xt = xt.reshape(NCORES, KR, NG4, 4, NQ, N).transpose(0, 1, 2, 4, 3, 5)
    xt = xt.reshape(NCORES, KR, NG4, NQ, 128)
    fv = filters.reshape(NCORES, PL, FH, FW, C, FOUT)
    ft = fv.transpose(0, 3, 4, 1, 2, 5).reshape(NCORES, KR, PL, NQ, FOUT)
    ft = ft.reshape(NCORES, KR, NG4, 4, NQ, FOUT).transpose(0, 1, 2, 4, 3, 5)
    ft = ft.reshape(NCORES, KR, NG4, NQ, 128)
    xfa = np.ascontiguousarray(
        np.concatenate([xt, ft], axis=4).astype(bf16)
    )
    bt = np.ascontiguousarray(
        np.concatenate([np.ones(128, np.float32), np.tile(bias, 16)])
        .astype(bf16)
        .reshape(1, 640)
    )
    return xfa, bt


def _assemble_bd(outs):
    """Per-core out [(s,b)=128, NG4, FOUT] -> full (N, 32, 32, FOUT)."""
    NG4 = PL // 4
    z = np.stack(outs)                                  # (core, (s,b), g, o)
    z = z.reshape(NCORES, 4, N, NG4, FOUT)              # (core, s, b, g, o)
    z = z.transpose(2, 0, 3, 1, 4)                      # (b, core, g, s, o)
    z = z.reshape(N, NCORES, PL, FOUT)                  # p_loc = 4*g + s
    return np.ascontiguousarray(z.reshape(N, NCORES * 4, 32, FOUT))


LAST_RESULT = None
VARIANT = "bd2"


def kernel(X, filters, bias):
    global LAST_RESULT
    from concourse import bass_utils
    from concourse.bass_utils import run_bass_kernel_spmd

    # If tracing is enabled in the environment, keep the artifact upload
    # local so a missing bucket can't fail the run.
    bass_utils.upload_artifacts = lambda tmpdir: f"local://{tmpdir}"

    if "nc" not in _CACHE:
        _CACHE["nc"] = {
            "fp32r": _build_module_r,
            "fp32": _build_module,
            "bf16ct": _build_module_bf16,
            "ct2": _build_module_ct2,
            "bd2": _build_module_bd2,
            "bd": _build_module_bd,
        }[VARIANT]()
    nc = _CACHE["nc"]
    if VARIANT == "bd":
        xfa, bt = _marshal_bd(X, filters, bias)
    elif VARIANT == "bd2":
        xfa, bt = _marshal_bd2(X, filters, bias)
    elif VARIANT in ("bf16ct", "ct2"):
        import ml_dtypes

        xfa, bt = _marshal(X, filters, bias, mdtype=ml_dtypes.bfloat16)
    else:
        xfa, bt = _marshal(X, filters, bias)
    if VARIANT == "fp32r":
        bt = np.ascontiguousarray(bt[:FOUT])
    in_maps = [{"xf": xfa[k], "bt": bt} for k in range(NCORES)]
    res = run_bass_kernel_spmd(nc, in_maps, core_ids=list(range(NCORES)))
    LAST_RESULT = res
    outs = [res.results[k]["out"] for k in range(NCORES)]
    if VARIANT == "bd":
        return _assemble_bd(outs)
    return _assemble_r(outs) if VARIANT == "fp32r" else _assemble(outs)



# revision 42
# speedup vs baseline: 1.1690x; 1.0815x over previous
"""Locally-connected conv (BioConvolution) Trainium2 kernel.

Problem: Z[n,p,o] = relu(sum_{ijc} patch[n,p,i,j,c] * filt[p,i,j,c,o] + bias[o])
  X: (32,128,128,32) f32, filters: (1024,4,4,32,32) f32, bias: (32,)
  out: (32,32,32,32) f32.   FH=FW=4 non-overlapping patches, P=1024.

Sharding: patch-parallel over P across 8 cores; core k owns patches
[128k,128k+128). Every input element is touched exactly once, so the
kernel is purely HBM-bound and the only big lever is BYTES: inputs are
cast to bf16 host-side (fp32 PSUM accumulation keeps rel err ~3e-3 vs
the 2e-2 gate), halving traffic to 8.4 MB in + 0.26 MB out per core
(~23.5 us at the ~375 GB/s per-core wall). Output is stored bf16 and
upcast on the host.

Shipped variant "bd2" (2-patch-block matmuls):
  - The tensor engine's sustained issue rate is ~34 ns/instruction (NX
    fetch-limited; bursts only drain the 64-deep queue, and the stream
    stalls ~2.3 us at each 256-instruction IRAM page boundary), so the
    naive 512 self-loading matmuls (1024 instructions) pace the kernel,
    not the FLOPs. Each matmul here covers TWO patches: stationary =
    filters [128, (p,o)=64], moving = X [128, (p',b)=64], out = [64,64]
    whose two diagonal 32x32 blocks are the wanted Z (half the MACs are
    discarded; the PE has 8x headroom). 256 matmuls = 512 instructions.
  - The two blocks of a 4-patch pair sit at PE column positions 0/64
    (tile_position) and their matmuls are emitted q-outer so they
    overlap in the array's column halves.
  - All 13 load chunks are armed up-front on the sync HWDGE ring (the
    whole 64 KB/partition input lives in SBUF, one buffer per chunk):
    arms carry no buffer-recycle waits, and completion-semaphore reuse
    only ever references an earlier load — never a store, whose packets
    would round-robin at ~6% bandwidth behind the load queue and stall
    everything (measured failure mode).
  - Diagonal extraction + bias + ReLU runs per 8-patch PSUM bank, split
    between ScalarE (ACT, lower block half) and the DVE
    (tensor_scalar add-bias/max-0, upper half), writing bf16 staging.
    Tile chains the cross-engine readers of each PSUM tile; that chain
    is load-bearing — the pool-recycle waits only name the scalar
    reader (verified in the BIR), and per-engine PSUM pools without the
    chain raced the DVE reader intermittently (NaNs).
  - Stores: bulk [0:28) groups on sync behind the load queue (packets
    flow the moment the stream drains), final [28:32) on the empty
    scalar ring so its descriptor fetch overlaps the bulk store.
Measured: ~38.7-42.0 us NEFF exec across runs (run-to-run device
variance), vs 63.5 us for the fp32r per-patch baseline. Floor is
~8.4 us fixed preamble (engine boot, barriers, descriptor prefetch)
+ ~23.9 us bf16 load stream at the HBM wall + ~3 us tail/teardown.
"""

import numpy as np

N, H, W, C = 32, 128, 128, 32
FH = FW = 4
FOUT = 32
NCORES = 8
PL = 128          # patches per core
NQ = 4            # K-chunks per patch (512 / 128)
KR = 128          # contraction rows per chunk (SBUF partitions)
NG = PL // 4      # 4-patch groups per core

_CACHE = {}


def _build_module(bufs=6, out_splits=8, mm_dtype="float32"):
    from concourse import bacc, tile, mybir

    nc = bacc.Bacc("TRN2", target_bir_lowering=False, debug=False, enable_asserts=False)
    dt = mybir.dt.float32
    mdt = getattr(mybir.dt, mm_dtype)
    # xf packs data and filters: [..., 0:32] = batch cols, [..., 32:64] = fout
    xf = nc.dram_tensor("xf", [KR, PL, NQ, N + FOUT], mdt, kind="ExternalInput").ap()
    bt = nc.dram_tensor("bt", [KR, 1], dt, kind="ExternalInput").ap()
    out = nc.dram_tensor("out", [KR, NG, N], dt, kind="ExternalOutput").ap()

    # Graduated chunk sizes (in patches): small first chunks so the first
    # matmul isn't gated on a full-size load sharing bandwidth round-robin.
    sizes = [2, 2, 4]
    rest = PL - sum(sizes)
    sizes += [8] * (rest // 8)
    assert sum(sizes) == PL
    GSPLIT = NG // out_splits
    relu = mybir.ActivationFunctionType.Relu

    with tile.TileContext(nc) as tc:
        with (
            tc.tile_pool(name="xfpool", bufs=bufs) as xfpool,
            tc.tile_pool(name="psum", bufs=8, space="PSUM") as psum,
            tc.tile_pool(name="misc", bufs=1) as misc,
        ):
            bias_t = misc.tile([KR, 1], dt)
            nc.sync.dma_start(bias_t[:], bt[:])
            staging = misc.tile([KR, NG, N], dt)

            p0 = 0
            for ch, PC in enumerate(sizes):
                xtile = xfpool.tile([KR, PC, NQ, N + FOUT], mdt, tag="xf")
                sl = slice(p0, p0 + PC)
                eng = nc.sync if ch % 2 == 0 else nc.scalar
                eng.dma_start(xtile[:], xf[:, sl, :, :])
                for g in range(PC // 2):
                    gg = (p0 + g * 2) // 4       # psum group id (2 patches/iter)
                    half = (p0 + g * 2) % 4      # 0 or 2: which half of the group
                    if half == 0:
                        ptile = psum.tile([KR, N], dt, tag="ps")
                    for s2 in range(2):
                        s = half + s2
                        p = g * 2 + s2
                        for q in range(NQ):
                            nc.tensor.matmul(
                                ptile[32 * s : 32 * s + 32, :],
                                xtile[:, p, q, N : N + FOUT],  # lhsT [128,32(o)]
                                xtile[:, p, q, 0:N],           # rhs  [128,32(b)]
                                start=(q == 0),
                                stop=(q == NQ - 1),
                                tile_position=(0, 32 * s),
                            )
                    if half == 2:
                        nc.scalar.activation(
                            staging[:, gg, :], ptile[:], relu, bias=bias_t[:]
                        )
                        if (gg + 1) % GSPLIT == 0:
                            osl = slice(gg + 1 - GSPLIT, gg + 1)
                            oeng = nc.sync if gg + 1 == NG else nc.gpsimd
                            oeng.dma_start(out[:, osl, :], staging[:, osl, :])
                p0 += PC
    nc.compile()
    return nc


def _build_module_r(bufs=8):
    """float32r variant: single-pass fp32 matmuls (tf32-ish precision),
    PSUM packing along the free axis (8 patches per bank) since fp32r
    requires dst base partition 0. Half the PE instruction stream of the
    fp32 variant -> fewer IRAM paging stalls."""
    from concourse import bacc, tile, mybir

    nc = bacc.Bacc("TRN2", target_bir_lowering=False, debug=False, enable_asserts=False)
    dt = mybir.dt.float32
    mdt = mybir.dt.float32r
    SG = 8                      # patches per PSUM super-group
    NSG = PL // SG              # 16
    xf = nc.dram_tensor("xf", [KR, PL, NQ, N + FOUT], mdt, kind="ExternalInput").ap()
    bt = nc.dram_tensor("bt", [FOUT, 1], dt, kind="ExternalInput").ap()
    out = nc.dram_tensor("out", [FOUT, PL, N], dt, kind="ExternalOutput").ap()

    # Graduated [2,2,4] head (earliest first matmul; measured tightest
    # variance) and a [4,4] tail that halves the final
    # load->matmul->ACT->store chain.
    sizes = [2, 2, 4] + [8] * ((PL - 16) // 8) + [4, 2, 2]
    assert sum(sizes) == PL
    # PSUM eviction groups: 8-patch banks, except two 4-patch mini-groups
    # at the end so the last matmul->ACT->store chain is half as long.
    groups = [(g * SG, SG) for g in range(NSG - 1)] + [(PL - 8, 4), (PL - 4, 4)]
    gof = {}
    for gi, (s0, gsz) in enumerate(groups):
        for i in range(gsz):
            gof[s0 + i] = (gi, i)
    relu = mybir.ActivationFunctionType.Relu

    with tile.TileContext(nc) as tc:
        with (
            tc.tile_pool(name="xfpool", bufs=bufs) as xfpool,
            tc.tile_pool(name="psum", bufs=6, space="PSUM") as psum,
            tc.tile_pool(name="misc", bufs=1) as misc,
        ):
            # bias rides the scalar ring so it doesn't burn sync's first
            # DMA slot (~0.7 us of stream start).
            bias_t = misc.tile([FOUT, 1], dt)
            nc.scalar.dma_start(bias_t[:], bt[:])
            staging = misc.tile([FOUT, PL, N], dt)

            p0 = 0
            ptile = None
            for ch, PC in enumerate(sizes):
                xtile = xfpool.tile([KR, PC, NQ, N + FOUT], mdt, tag="xf")
                # All loads on sync's single HWDGE FIFO: strictly in-order
                # completions. (Arming chunk 0 on the scalar ring was tried
                # and is bimodal: when sync's big queue gets ahead, chunk 0
                # drains at round-robin half-rate and the in-order PE
                # consumption slips ~8 us.)
                nc.sync.dma_start(xtile[:], xf[:, p0 : p0 + PC, :, :])
                for pl in range(PC):
                    p = p0 + pl
                    gi, i = gof[p]
                    s0, gsz = groups[gi]
                    if i == 0:
                        ptile = psum.tile([FOUT, SG, N], dt, tag="ps")
                    for q in range(NQ):
                        nc.tensor.matmul(
                            ptile[:, i, :],
                            xtile[:, pl, q, N : N + FOUT],  # lhsT [128,32(o)]
                            xtile[:, pl, q, 0:N],           # rhs  [128,32(b)]
                            start=(q == 0),
                            stop=(q == NQ - 1),
                        )
                    if i == gsz - 1:
                        nc.scalar.activation(
                            staging[:, s0 : s0 + gsz, :],
                            ptile[:, :gsz, :],
                            relu,
                            bias=bias_t[:],
                        )
                        # Stores also ride the scalar ring, LAGGED two groups
                        # behind the ACT stream: their ACT dependency is long
                        # complete, so they never stall scalar (and the sync
                        # load ring is untouched). The final two stores are
                        # pure program-order after the last ACT.
                        if gi == len(groups) - 1:
                            a = groups[gi - 2][0]
                            nc.scalar.dma_start(
                                out[:, a:s0, :], staging[:, a:s0, :]
                            )
                            nc.scalar.dma_start(
                                out[:, s0:PL, :], staging[:, s0:PL, :]
                            )
                        elif gi % 2 == 1 and gi >= 3:
                            a = groups[gi - 3][0]
                            b = groups[gi - 1][0]
                            nc.scalar.dma_start(
                                out[:, a:b, :], staging[:, a:b, :]
                            )
                p0 += PC
    nc.compile()
    return nc


def _build_module_bf16(bufs=6):
    """bf16 variant: inputs cast to bf16 host-side (halves HBM traffic, the
    true bottleneck; fp32 PSUM accumulation keeps rel err ~1e-3). Matmuls
    are column-tiled: 4 patches' [128,32] filter blocks sit on the 4
    column strips of the PE array via tile_position=(0,32s), emitted
    q-outer so the 4 strips run concurrently (per-subarray concurrency)
    and the PE stream can't pace the now-faster load stream."""
    from concourse import bacc, tile, mybir

    nc = bacc.Bacc("TRN2", target_bir_lowering=False, debug=False, enable_asserts=False)
    dt = mybir.dt.float32
    mdt = mybir.dt.bfloat16
    xf = nc.dram_tensor("xf", [KR, PL, NQ, N + FOUT], mdt, kind="ExternalInput").ap()
    bt = nc.dram_tensor("bt", [KR, 1], dt, kind="ExternalInput").ap()
    # Output stored bf16 (host upcasts): halves store traffic and staging.
    out = nc.dram_tensor("out", [KR, NG, N], mdt, kind="ExternalOutput").ap()

    # Graduated head (earliest first matmul) and a short tail that
    # shortens the final load->matmul->ACT->store chain. 16-patch main
    # chunks keep per-arm transfer (~1.4 us) above the sync engine's
    # ~0.65 us arm rate and halve descriptor volume vs 8.
    sizes = [2, 2, 4, 8] + [16] * 6 + [8, 4, 2, 2]
    assert sum(sizes) == PL
    # ACT groups: 8 patches (two 4-patch col-tile passes side by side on
    # the PSUM free axis), except two 4-patch mini-groups at the end.
    groups = [(g * 8, 8) for g in range((PL - 8) // 8)] + [(PL - 8, 4), (PL - 4, 4)]
    gof = {}
    for gi, (s0, gsz) in enumerate(groups):
        for i in range(gsz):
            gof[s0 + i] = gi
    relu = mybir.ActivationFunctionType.Relu

    with tile.TileContext(nc) as tc:
        with (
            # The whole per-core input (64 KB/partition) fits in SBUF, so
            # every chunk gets its own buffer: load arms carry no
            # buffer-recycle waits and all issue back-to-back at the start.
            tc.tile_pool(name="xfpool", bufs=len(sizes)) as xfpool,
            # One PSUM tile per ACT group (17 x 256B/partition): no PSUM
            # recycling, so matmul groups never stall on an ACT 8 groups
            # back (the recycle coupling cost multi-us hiccups).
            tc.tile_pool(name="psum", bufs=17, space="PSUM") as psum,
            tc.tile_pool(name="misc", bufs=1) as misc,
        ):
            bias_t = misc.tile([KR, 1], dt)
            nc.scalar.dma_start(bias_t[:], bt[:])
            staging = misc.tile([KR, NG, N], mdt)

            # All load arms first, in program order: any completion-sem
            # reuse then only ever waits on an earlier LOAD (long done) —
            # never on a store, whose packets drain at a trickle behind
            # the load queue and would stall the whole load stream.
            xtiles = []
            p0 = 0
            for PC in sizes:
                xtile = xfpool.tile([KR, PC, NQ, N + FOUT], mdt, tag="xf")
                nc.sync.dma_start(xtile[:], xf[:, p0 : p0 + PC, :, :])
                xtiles.append((p0, PC, xtile))
                p0 += PC

            ptile = None
            for p0, PC, xtile in xtiles:
                # Matmuls go q-outer within each 4-patch col-tile pass so
                # consecutive instructions hit different column strips.
                for b0 in range(0, PC, 4):
                    npass = min(4, PC - b0)        # patches in this pass
                    p = p0 + b0
                    gi = gof[p]
                    s0, gsz = groups[gi]
                    j = (p - s0) // 4              # free-axis slot in psum tile
                    if p == s0:
                        ptile = psum.tile([KR, gsz // 4, N], dt, tag="ps")
                    for q in range(NQ):
                        for si in range(npass):
                            s = (p + si - s0) % 4
                            nc.tensor.matmul(
                                ptile[32 * s : 32 * s + 32, j, :],
                                xtile[:, b0 + si, q, N : N + FOUT],
                                xtile[:, b0 + si, q, 0:N],
                                start=(q == 0),
                                stop=(q == NQ - 1),
                                tile_position=(0, 32 * s),
                            )
                    if p + npass == s0 + gsz:
                        g4 = s0 // 4
                        ng4 = gsz // 4
                        nc.scalar.activation(
                            staging[:, g4 : g4 + ng4, :],
                            ptile[:, :ng4, :],
                            relu,
                            bias=bias_t[:],
                        )
                        # Two stores at the end, on sync's queue behind the
                        # loads (in-order: packets flow the moment the load
                        # stream drains). Arm the bulk store a few groups
                        # early so its descriptor fetch overlaps the
                        # remaining loads; the last arm covers only the
                        # final 4 groups to keep the tail chain short.
                        if gi == len(groups) - 4:
                            a = g4 + ng4
                            nc.sync.dma_start(
                                out[:, 0:a, :], staging[:, 0:a, :]
                            )
                        elif gi == len(groups) - 1:
                            a = groups[len(groups) - 4][0] // 4 + 2
                            e = PL // 4
                            nc.sync.dma_start(
                                out[:, a:e, :], staging[:, a:e, :]
                            )
    nc.compile()
    return nc


def _build_module_ct2():
    """Shared-LDWEIGHTS col-tiled variant. The tensor engine's sustained
    issue rate is ~34 ns/instruction (NX fetch-limited; bursts only drain
    the 64-deep queue), so the 1024-instruction LDW+MM stream of the
    per-patch variant paces the whole kernel. Here ONE 128-column
    LDWEIGHTS per (4-patch group, q) loads all 4 strips' filters and the
    4 matmuls are emitted non-self-loading (InstMatmult.ldweights=False):
    640 tensor instructions. A ~3.4 us warm-up primer of dummy matmuls at
    program start flips the PE's HAM clock gate to 2.4 GHz before the
    real stream begins."""
    from concourse import bacc, tile, mybir

    nc = bacc.Bacc("TRN2", target_bir_lowering=False, debug=False, enable_asserts=False)
    dt = mybir.dt.float32
    mdt = mybir.dt.bfloat16
    xf = nc.dram_tensor("xf", [KR, PL, NQ, N + FOUT], mdt, kind="ExternalInput").ap()
    bt = nc.dram_tensor("bt", [KR, 1], dt, kind="ExternalInput").ap()
    out = nc.dram_tensor("out", [KR, NG, N], mdt, kind="ExternalOutput").ap()

    sizes = [2, 2, 4, 8] + [16] * 6 + [8, 4, 2, 2]
    assert sum(sizes) == PL
    groups = [(g * 8, 8) for g in range((PL - 8) // 8)] + [(PL - 8, 4), (PL - 4, 4)]
    gof = {}
    for gi, (s0, gsz) in enumerate(groups):
        for i in range(gsz):
            gof[s0 + i] = gi
    relu = mybir.ActivationFunctionType.Relu

    with tile.TileContext(nc) as tc:
        with (
            tc.tile_pool(name="xfpool", bufs=len(sizes)) as xfpool,
            tc.tile_pool(name="psum", bufs=7, space="PSUM") as psum,
            tc.tile_pool(name="pscr", bufs=1, space="PSUM") as pscr,
            tc.tile_pool(name="misc", bufs=1) as misc,
        ):
            bias_t = misc.tile([KR, 1], dt)
            nc.scalar.dma_start(bias_t[:], bt[:])
            staging = misc.tile([KR, NG, N], mdt)

            # HAM warm-up primer: ~9 zero x zero matmuls keep the PE array
            # busy from program start (~6.7 us) until real data arrives
            # (~10.3 us) so the clock gate is at 8/8 for the whole stream.
            scratch = misc.tile([KR, 640], mdt)
            nc.gpsimd.memset(scratch[:], 0)
            pdummy = pscr.tile([KR, 512], dt)
            for _ in range(9):
                nc.tensor.matmul(
                    pdummy[:],
                    scratch[:, 512:640],
                    scratch[:, 0:512],
                    start=True,
                    stop=True,
                    skip_group_check=True,
                )

            xtiles = []
            p0 = 0
            for PC in sizes:
                xtile = xfpool.tile([KR, PC, NQ, N + FOUT], mdt, tag="xf")
                nc.sync.dma_start(xtile[:], xf[:, p0 : p0 + PC, :, :])
                xtiles.append((p0, PC, xtile))
                p0 += PC

            ptile = None
            for p0, PC, xtile in xtiles:
                for b0 in range(0, PC, 4):
                    npass = min(4, PC - b0)
                    p = p0 + b0
                    gi = gof[p]
                    s0, gsz = groups[gi]
                    j = (p - s0) // 4
                    if p == s0:
                        ptile = psum.tile([KR, gsz // 4, N], dt, tag="ps")
                    s_start = (p - s0) % 4
                    for q in range(NQ):
                        # One LDW covers this pass's patches across the
                        # column strips (columns 32*s_start + p_local*32+o).
                        nc.tensor.ldweights(
                            xtile[:, b0 : b0 + npass, q, N : N + FOUT],
                            tile_position=(0, 32 * s_start),
                        )
                        for si in range(npass):
                            s = (p + si - s0) % 4
                            mm = nc.tensor.matmul(
                                ptile[32 * s : 32 * s + 32, j, :],
                                xtile[:, b0 + si, q, N : N + FOUT],
                                xtile[:, b0 + si, q, 0:N],
                                start=(q == 0),
                                stop=(q == NQ - 1),
                                tile_position=(0, 32 * s),
                                skip_group_check=True,
                            )
                            mm.ins.ldweights = False
                    if p + npass == s0 + gsz:
                        g4 = s0 // 4
                        ng4 = gsz // 4
                        nc.scalar.activation(
                            staging[:, g4 : g4 + ng4, :],
                            ptile[:, :ng4, :],
                            relu,
                            bias=bias_t[:],
                        )
                        if gi == len(groups) - 4:
                            a = g4 + ng4
                            nc.sync.dma_start(
                                out[:, 0:a, :], staging[:, 0:a, :]
                            )
                        elif gi == len(groups) - 1:
                            a = groups[len(groups) - 4][0] // 4 + 2
                            e = PL // 4
                            nc.sync.dma_start(
                                out[:, a:e, :], staging[:, a:e, :]
                            )
    nc.compile()
    return nc


def _build_module_bd2():
    """2-patch-block variant. The tensor engine's sustained issue rate is
    ~34 ns/instruction (NX fetch-limited), so the 1024-instruction
    per-patch stream (512 self-loading matmuls) paces the kernel at
    ~35 us. Here each matmul covers TWO patches: stationary = filters of
    2 patches [128, (p,o)=64], moving = X of 2 patches [128, (p',b)=64],
    out = [64, 64] of which the two diagonal 32x32 blocks are wanted Z.
    Two blocks per 4-patch group sit at column positions 0/64 and overlap
    in the array. 256 matmuls = 512 tensor fetches. Bias+ReLU runs in the
    diagonal extraction: scalar ACT (bias AP + Relu) for the lower half,
    DVE tensor_scalar(add bias, max 0) for the upper half."""
    from concourse import bacc, tile, mybir

    nc = bacc.Bacc("TRN2", target_bir_lowering=False, debug=False, enable_asserts=False)
    dt = mybir.dt.float32
    mdt = mybir.dt.bfloat16
    NB = PL // 2                       # 2-patch blocks per core
    # xf cols per (block, q): [0:64] = X of the 2 patches ((w,b) flat),
    # [64:128] = filters ((w,o) flat) — contiguous, single-free-dim APs.
    xf = nc.dram_tensor("xf", [KR, NB, NQ, 128], mdt, kind="ExternalInput").ap()
    bt = nc.dram_tensor("bt", [KR, 1], dt, kind="ExternalInput").ap()
    out = nc.dram_tensor("out", [KR, NG, N], mdt, kind="ExternalOutput").ap()

    # 11 chunks: fewer arms = less completion-semaphore reuse pressure
    # (the pool has ~10; deep reuse chains stalled arms in some runs) and
    # fewer descriptor fetches on the E64 engine.
    sizes = [1, 1, 2, 4, 8, 12, 12, 12, 8, 2, 2]    # in blocks
    assert sum(sizes) == NB
    # PSUM groups of 4 blocks (8 patches, one bank), two 2-block groups
    # at the end for a short final chain. The pool-recycle waits only
    # name the scalar reader, but the DVE reader is chained ~one
    # instruction behind scalar by Tile's cross-engine accessor ordering
    # and the recycle distance is 8 groups (~17 us of DMA) — safe margin.
    groups = [(g * 4, 4) for g in range((NB - 4) // 4)] + [(NB - 4, 2), (NB - 2, 2)]
    gof = {}
    for gi, (b0, gsz) in enumerate(groups):
        for i in range(gsz):
            gof[b0 + i] = gi
    relu = mybir.ActivationFunctionType.Relu
    add_op = mybir.AluOpType.add
    max_op = mybir.AluOpType.max

    with tile.TileContext(nc) as tc:
        with (
            tc.tile_pool(name="xfpool", bufs=len(sizes)) as xfpool,
            # ONE psum pool shared by both extraction engines. Tile chains
            # cross-engine readers of a pooled tile (serializing vector
            # extractions ~300ns behind scalar's) — that chain is what
            # makes the pool-recycle waits sound: with per-engine pools the
            # recycled banks' matmuls carried NO wait on the DVE reader
            # (verified in the IR) and intermittently raced it.
            tc.tile_pool(name="psum", bufs=8, space="PSUM") as psum,
            tc.tile_pool(name="misc", bufs=1) as misc,
        ):
            bias_t = misc.tile([KR, 1], dt)
            nc.scalar.dma_start(bias_t[:], bt[:])
            staging = misc.tile([KR, NG, N], mdt)

            xtiles = []
            b0 = 0
            for BC in sizes:
                xtile = xfpool.tile([KR, BC, NQ, 128], mdt, tag="xf")
                nc.sync.dma_start(xtile[:], xf[:, b0 : b0 + BC, :, :])
                xtiles.append((b0, BC, xtile))
                b0 += BC

            ptile = None
            for c0, BC, xtile in xtiles:
                for j0 in range(0, BC, 2):
                    npair = min(2, BC - j0)
                    blk0 = c0 + j0
                    gi = gof[blk0]
                    g0, gsz = groups[gi]
                    j = (blk0 - g0) // 2           # psum free slot (pair idx)
                    if blk0 == g0:
                        ptile = psum.tile([KR, gsz // 2, 64], dt, tag="ps")
                    # q-outer, block-inner: the pair's matmuls overlap in
                    # the array (column positions 0/64) so the chain is
                    # ~half as long as two serial per-block chains.
                    for q in range(NQ):
                        for h in range(npair):
                            blk = blk0 + h
                            off = 64 * (blk % 2)   # block position: 0 or 64
                            nc.tensor.matmul(
                                ptile[off : off + 64, j, :],
                                xtile[:, j0 + h, q, 64:128],  # stationary: filters
                                xtile[:, j0 + h, q, 0:64],    # moving: X
                                start=(q == 0),
                                stop=(q == NQ - 1),
                                tile_position=(0, off),
                                skip_group_check=True,
                            )
                    blk = blk0 + npair - 1
                    if blk == g0 + gsz - 1:
                        g4 = g0 // 2
                        ng4 = gsz // 2
                        # DVE (upper half) first, scalar second: Tile chains
                        # cross-engine readers in emission order, so the
                        # chain tail is the scalar ACT — the scalar-armed
                        # final store then follows in pure program order.
                        for h in (1, 0):
                            for w in range(2):
                                pl4 = 2 * h + w
                                pr = 64 * h + 32 * w
                                src = ptile[pr : pr + 32, :ng4, 32 * w : 32 * w + 32]
                                dst = staging[32 * pl4 : 32 * pl4 + 32, g4 : g4 + ng4, :]
                                bsl = bias_t[pr : pr + 32]
                                if h == 0:
                                    nc.scalar.activation(dst, src, relu, bias=bsl)
                                else:
                                    nc.vector.tensor_scalar(
                                        dst, src, bsl, 0.0, add_op, max_op
                                    )
                        if gi == len(groups) - 4:
                            a = g4 + ng4
                            nc.sync.dma_start(
                                out[:, 0:a, :], staging[:, 0:a, :]
                            )
                        elif gi == len(groups) - 1:
                            # Final store on the scalar ring: its queue is
                            # empty, so descriptor fetch and packets overlap
                            # the bulk store draining behind Q1.
                            a = groups[len(groups) - 4][0] // 2 + 2
                            e = PL // 4
                            nc.scalar.dma_start(
                                out[:, a:e, :], staging[:, a:e, :]
                            )
    nc.compile()
    return nc


def _build_module_bd():
    """Block-diagonal variant: per 4-patch group and K-chunk q, ONE
    128-column LDWEIGHTS loads the 4 patches' X [128, (p,b)=128] as the
    stationary operand and ONE matmul streams the 4 patches' filters
    [128, (p',o)=128] as the moving operand, computing all 16 cross
    blocks out[(p,b),(p',o)] of which the 4 diagonal p==p' blocks are the
    wanted Z. 8 tensor instructions per group (~280 total) instead of 32
    (~1024): the tensor stream was issue-limited at ~34 ns/instruction,
    not FLOP-limited, and 4x fewer instructions takes it off the critical
    path. Bias is folded in as a K=1 rank-one matmul (ones x bias) so the
    diagonal extraction is a pure ReLU, split across the scalar AND
    vector engines (2 strips each)."""
    from concourse import bacc, tile, mybir

    nc = bacc.Bacc("TRN2", target_bir_lowering=False, debug=False, enable_asserts=False)
    dt = mybir.dt.float32
    mdt = mybir.dt.bfloat16
    NG4 = PL // 4                      # 4-patch groups per core
    # xf2 cols: [0:128] = X block (p_local*32+b), [128:256] = filter block
    # (p_local*32+o), both contiguous so LDWEIGHTS gets FWL and the moving
    # stream is a single run.
    xf = nc.dram_tensor("xf", [KR, NG4, NQ, 256], mdt, kind="ExternalInput").ap()
    # bt: [0:128] = ones (K=1 stationary for the bias matmul),
    # [128:640] = bias tiled 16x (moving operand, 128 per group slot).
    bt = nc.dram_tensor("bt", [1, 640], mdt, kind="ExternalInput").ap()
    out = nc.dram_tensor("out", [KR, NG4, FOUT], dt, kind="ExternalOutput").ap()

    # Chunk sizes in 4-patch groups; short head and tail chunks keep the
    # first matmul early and the final load->mm->relu->store chain short.
    sizes = [1, 1, 2] + [4] * 6 + [2, 1, 1]
    assert sum(sizes) == NG4
    relu = mybir.ActivationFunctionType.Relu

    with tile.TileContext(nc) as tc:
        with (
            tc.tile_pool(name="xfpool", bufs=len(sizes)) as xfpool,
            tc.tile_pool(name="psum", bufs=4, space="PSUM") as psum,
            tc.tile_pool(name="misc", bufs=1) as misc,
        ):
            bias_t = misc.tile([1, 640], mdt)
            nc.scalar.dma_start(bias_t[:], bt[:])
            staging = misc.tile([KR, NG4, FOUT], dt)

            # All load arms first (whole input resident in SBUF): no
            # buffer-recycle waits, and completion-sem reuse only ever
            # references an earlier load.
            xtiles = []
            g0 = 0
            for GC in sizes:
                xtile = xfpool.tile([KR, GC, NQ, 256], mdt, tag="xf")
                nc.sync.dma_start(xtile[:], xf[:, g0 : g0 + GC, :, :])
                xtiles.append((g0, GC, xtile))
                g0 += GC

            for ci, (g0, GC, xtile) in enumerate(xtiles):
                ptile = psum.tile([KR, GC, 128], dt, tag="ps")
                for j in range(GC):
                    for q in range(NQ):
                        nc.tensor.matmul(
                            ptile[:, j, :],
                            xtile[:, j, q, 0:128],      # stationary: X
                            xtile[:, j, q, 128:256],    # moving: filters
                            start=(q == 0),
                            stop=False,
                            skip_group_check=True,
                        )
                    # K=1 matmul adds bias[o] to every column of region j.
                    # Must come before the NEXT region's start=True matmul:
                    # start clears has_written beyond its own out region,
                    # and a start=False write onto cleared bits OVERWRITES
                    # instead of accumulating.
                    nc.tensor.matmul(
                        ptile[:, j, :],
                        bias_t[:, 0:128],
                        bias_t[:, 128:256],
                        start=False,
                        stop=True,
                        skip_group_check=True,
                    )
                # Diagonal extraction + ReLU, 2 strips on scalar, 2 on DVE.
                for s in range(4):
                    src = ptile[32 * s : 32 * s + 32, :, 32 * s : 32 * s + 32]
                    dst = staging[32 * s : 32 * s + 32, g0 : g0 + GC, :]
                    if s < 2:
                        nc.scalar.activation(dst, src, relu)
                    else:
                        nc.vector.tensor_scalar_max(dst, src, 0.0)
                if ci == len(sizes) - 2:
                    a = g0 + GC
                    nc.sync.dma_start(out[:, 0:a, :], staging[:, 0:a, :])
                elif ci == len(sizes) - 1:
                    a = g0
                    nc.sync.dma_start(out[:, a:NG4, :], staging[:, a:NG4, :])
    nc.compile()
    return nc


def _get_module():
    if "nc" not in _CACHE:
        _CACHE["nc"] = _build_module()
    return _CACHE["nc"]


def _marshal(X, filters, bias, mdtype=np.float32):
    """Shard + lay out full inputs into per-core device arrays."""
    X = np.ascontiguousarray(np.asarray(X, dtype=np.float32))
    filters = np.ascontiguousarray(np.asarray(filters, dtype=np.float32))
    bias = np.asarray(bias, dtype=np.float32)

    # X: (b, core, pr, i, pc, j, c) -> (core, j, c, pr, pc, i, b)
    xv = X.reshape(N, NCORES, 4, FH, 32, FW, C)
    xt = xv.transpose(1, 5, 6, 2, 4, 3, 0).reshape(NCORES, KR, PL, NQ, N)
    # filters: (core, p, i, j, c, o) -> (core, j, c, p, i, o)
    fv = filters.reshape(NCORES, PL, FH, FW, C, FOUT)
    ft = fv.transpose(0, 3, 4, 1, 2, 5).reshape(NCORES, KR, PL, NQ, FOUT)
    xfa = np.concatenate([xt, ft], axis=4)
    if xfa.dtype != mdtype:
        xfa = xfa.astype(mdtype)  # round-to-nearest-even
    xfa = np.ascontiguousarray(xfa)
    bt = np.ascontiguousarray(np.tile(bias, 4).reshape(KR, 1))
    return xfa, bt


def _assemble(outs):
    """Per-core out [128=(s,o), NG, N] -> full (N, 32, 32, FOUT)."""
    z = np.stack(outs).astype(np.float32)               # (core, (s,o), g, b)
    z = z.reshape(NCORES, 4, FOUT, NG, N)               # (core, s, o, g, b)
    z = z.transpose(4, 0, 3, 1, 2)                      # (b, core, g, s, o)
    z = z.reshape(N, NCORES, PL, FOUT)                  # p_loc = 4*g + s
    z = z.reshape(N, NCORES * 4, 32, FOUT)              # (b, pr_glob, pc, o)
    return np.ascontiguousarray(z)


def _assemble_r(outs):
    """Per-core out [FOUT, PL, N] -> full (N, 32, 32, FOUT)."""
    z = np.stack(outs)                                  # (core, o, p, b)
    z = z.transpose(3, 0, 2, 1)                         # (b, core, p, o)
    return np.ascontiguousarray(z.reshape(N, 32, 32, FOUT))


def _marshal_bd2(X, filters, bias):
    """Pair-blocked layout for bd2: xf[r, blk, q, 0:64] = X of block blk's
    2 patches ((w,b) flattened), [.., 64:128] = the matching filter block
    ((w,o) flattened); both contiguous (single-free-dim matmul APs)."""
    import ml_dtypes

    bf16 = ml_dtypes.bfloat16
    NB = PL // 2
    X = np.ascontiguousarray(np.asarray(X, dtype=np.float32))
    filters = np.ascontiguousarray(np.asarray(filters, dtype=np.float32))
    bias = np.asarray(bias, dtype=np.float32)

    xv = X.reshape(N, NCORES, 4, FH, 32, FW, C)
    xt = xv.transpose(1, 5, 6, 2, 4, 3, 0).reshape(NCORES, KR, PL, NQ, N)
    xt = xt.reshape(NCORES, KR, NB, 2, NQ, N).transpose(0, 1, 2, 4, 3, 5)
    xt = xt.reshape(NCORES, KR, NB, NQ, 64)
    fv = filters.reshape(NCORES, PL, FH, FW, C, FOUT)
    ft = fv.transpose(0, 3, 4, 1, 2, 5).reshape(NCORES, KR, PL, NQ, FOUT)
    ft = ft.reshape(NCORES, KR, NB, 2, NQ, FOUT).transpose(0, 1, 2, 4, 3, 5)
    ft = ft.reshape(NCORES, KR, NB, NQ, 64)
    xfa = np.ascontiguousarray(np.concatenate([xt, ft], axis=4).astype(bf16))
    bt = np.ascontiguousarray(np.tile(bias, 4).reshape(KR, 1))
    return xfa, bt


def _marshal_bd(X, filters, bias):
    """Block-diagonal layout: xf2[r, g, q, 0:128] = X of group g's 4
    patches ((p_local, b) flattened), [.., 128:256] = the matching filter
    block ((p_local, o) flattened); bt = [ones(128), tile(bias, 16)]."""
    import ml_dtypes

    bf16 = ml_dtypes.bfloat16
    X = np.ascontiguousarray(np.asarray(X, dtype=np.float32))
    filters = np.ascontiguousarray(np.asarray(filters, dtype=np.float32))
    bias = np.asarray(bias, dtype=np.float32)

    NG4 = PL // 4
    # X: (b, core, pr, i, pc, j, c) -> (core, j, c, pr, pc, i, b)
    xv = X.reshape(N, NCORES, 4, FH, 32, FW, C)
    xt = xv.transpose(1, 5, 6, 2, 4, 3, 0).reshape(NCORES, KR, PL, NQ, N)
    # -> (core, r, g, q, p_local, b)
    xt = xt.reshape(NCORES, KR, NG4, 4, NQ, N).transpose(0, 1, 2, 4, 3, 5)
    xt = xt.reshape(NCORES, KR, NG4, NQ, 128)
    fv = filters.reshape(NCORES, PL, FH, FW, C, FOUT)
    ft = fv.transpose(0, 3, 4, 1, 2, 5).reshape(NCORES, KR, PL, NQ, FOUT)
    ft = ft.reshape(NCORES, KR, NG4, 4, NQ, FOUT).transpose(0, 1, 2, 4, 3, 5)
    ft = ft.reshape(NCORES, KR, NG4, NQ, 128)
    xfa = np.ascontiguousarray(
        np.concatenate([xt, ft], axis=4).astype(bf16)
    )
    bt = np.ascontiguousarray(
        np.concatenate([np.ones(128, np.float32), np.tile(bias, 16)])
        .astype(bf16)
        .reshape(1, 640)
    )
    return xfa, bt


def _assemble_bd(outs):
    """Per-core out [(s,b)=128, NG4, FOUT] -> full (N, 32, 32, FOUT)."""
    NG4 = PL // 4
    z = np.stack(outs)                                  # (core, (s,b), g, o)
    z = z.reshape(NCORES, 4, N, NG4, FOUT)              # (core, s, b, g, o)
    z = z.transpose(2, 0, 3, 1, 4)                      # (b, core, g, s, o)
    z = z.reshape(N, NCORES, PL, FOUT)                  # p_loc = 4*g + s
    return np.ascontiguousarray(z.reshape(N, NCORES * 4, 32, FOUT))


LAST_RESULT = None
VARIANT = "bd2"


def kernel(X, filters, bias):
    global LAST_RESULT
    from concourse import bass_utils
    from concourse.bass_utils import run_bass_kernel_spmd

    # If tracing is enabled in the environment, keep the artifact upload
    # local so a missing bucket can't fail the run.
    bass_utils.upload_artifacts = lambda tmpdir: f"local://{tmpdir}"

    if "nc" not in _CACHE:
        _CACHE["nc"] = {
            "fp32r": _build_module_r,
            "fp32": _build_module,
            "bf16ct": _build_module_bf16,
            "ct2": _build_module_ct2,
            "bd2": _build_module_bd2,
            "bd": _build_module_bd,
        }[VARIANT]()
    nc = _CACHE["nc"]
    if VARIANT == "bd":
        xfa, bt = _marshal_bd(X, filters, bias)
    elif VARIANT == "bd2":
        xfa, bt = _marshal_bd2(X, filters, bias)
    elif VARIANT in ("bf16ct", "ct2"):
        import ml_dtypes

        xfa, bt = _marshal(X, filters, bias, mdtype=ml_dtypes.bfloat16)
    else:
        xfa, bt = _marshal(X, filters, bias)
    if VARIANT == "fp32r":
        bt = np.ascontiguousarray(bt[:FOUT])
    in_maps = [{"xf": xfa[k], "bt": bt} for k in range(NCORES)]
    res = run_bass_kernel_spmd(nc, in_maps, core_ids=list(range(NCORES)))
    LAST_RESULT = res
    outs = [res.results[k]["out"] for k in range(NCORES)]
    if VARIANT == "bd":
        return _assemble_bd(outs)
    return _assemble_r(outs) if VARIANT == "fp32r" else _assemble(outs)

